# revision 22
# baseline (speedup 1.0000x reference)
"""Trainium2 Bass kernel for nn_AttentionGCNLayer (B=2, N=4096, D=256, H=2, ITERS=2).

Sharding: 8 cores = (b in 2) x (h in 2) x (row-half in 2). Each core handles one
(batch, head) pair and one half (2048) of the attention rows, with a pairwise
AllGather of the updated node features between the two GCN iterations.

Schedule (v2): chunk-paced pipeline. The Scalar engine's exp stream (64
activations of [128,1024], ~73us) is the second wall after the PE (~114us);
the program interleaves PE work (scores, h-gen, R rowsums, agg accumulation)
at neighbor-chunk granularity so the PE is never idle waiting on exp:

  A:  kq gen + scores(mega-tile 0) exp-paced, h1 + R(rt0) + agg(rt0)
      partial accumulation interleaved per chunk-pair.
  B:  scores(mega-tile 1) feeding exp(1), R(rt1)/agg(rt1) blocks, then
      chunk-paced R(rt2)/agg(rt2) against the exp(1) drain; h2 + AllGather
      of updated features fire per row-pair as soon as x1 is ready.
  D:  iter-2 aggregation (pair order: cc0-half first, local, cc1-half last
      to match AllGather arrival), output projection + DMA interleaved.

Layout: x kept transposed (x^T [D on 2x128 partitions, N free]) in local row
order; scores computed transposed (E^T = exp(q k^T)^T, neighbors on
partitions) feeding the aggregation matmuls directly; E and h in fp8 with
DoubleRow matmuls; weights arrive pre-cast to bf16 from the host in a single
packed blob (no on-device staging casts). Softmax normalizer R = rowsum(E)
via DoubleRow ones-matmuls; 1/R via the fast DVE reciprocal. Scalar engine
runs exp only; copies/relu/bias live on the Vector engine.
"""

import sys

if "/opt/trn_rl_repo" not in sys.path:
    sys.path.insert(0, "/opt/trn_rl_repo")

import numpy as np

B, N, D, H, ITERS = 2, 4096, 256, 2, 2
DK = D // H                      # 128
RH = N // 2                      # 2048 rows per core
NCH = N // 128                   # 32 neighbor chunks
NCP = NCH // 2                   # 16 neighbor chunk-pairs
RT = 512                         # row tile (one PSUM bank of fp32)
NRT = RH // RT                   # 4 row tiles per core
SCALE = 1.0 / float(np.sqrt(np.float32(DK)))

# packed bf16 weight blob column offsets
WQ0, WK0, GW0, AGG0 = 0, 256, 512, 1536
WBCOLS = 2048

_CACHE = {}


def _seq_engines(mybir):
    return {
        mybir.EngineType.PE,
        mybir.EngineType.Activation,
        mybir.EngineType.Pool,
        mybir.EngineType.DVE,
        mybir.EngineType.SP,
    }


def _split_excess_waits(nc, mybir, max_waits=1):
    """This container's walrus accepts at most one sync-wait per engine
    instruction; hoist extra waits onto preceding NoOps on the same engine."""
    seq = _seq_engines(mybir)
    n_new = 0
    for f in nc.m.functions:
        for blk in f.blocks:
            if not any(
                inst.sync_info is not None
                and inst.sync_info.on_wait
                and len(inst.sync_info.on_wait) > max_waits
                and inst.engine in seq
                for inst in blk.instructions
            ):
                continue
            out = []
            for inst in blk.instructions:
                si = inst.sync_info
                if (
                    si is not None
                    and si.on_wait
                    and len(si.on_wait) > max_waits
                    and inst.engine in seq
                ):
                    waits = list(si.on_wait)
                    keep, extra = waits[:max_waits], waits[max_waits:]
                    while extra:
                        chunk, extra = extra[:max_waits], extra[max_waits:]
                        out.append(
                            mybir.InstNoOp(
                                name=f"{inst.name}-ws{n_new}",
                                sync_info=mybir.SyncInfo(on_wait=chunk, on_update=[]),
                                bass_nofuse=True,
                                engine=inst.engine,
                            )
                        )
                        n_new += 1
                    inst.sync_info = mybir.SyncInfo(
                        on_wait=keep, on_update=list(si.on_update)
                    )
                out.append(inst)
            blk.instructions = out
    return n_new


def _build():
    import concourse.bass as bass
    import concourse.mybir as mybir
    import concourse.tile as tile

    f32 = mybir.dt.float32
    bf16 = mybir.dt.bfloat16
    fp8 = mybir.dt.float8e4
    AF = mybir.ActivationFunctionType
    ALU = mybir.AluOpType

    nc = bass.Bass("TRN2", num_devices=8)

    nodes = nc.dram_tensor("nodes", [D, N], bf16, kind="ExternalInput")
    wb = nc.dram_tensor("wb", [128, WBCOLS], bf16, kind="ExternalInput")
    fb = nc.dram_tensor("fb", [128, 8], f32, kind="ExternalInput")
    part = nc.dram_tensor("part", [RH, D], f32, kind="ExternalOutput")

    with tile.TileContext(nc) as tc:
        from contextlib import ExitStack

        with ExitStack() as ctx:
            const = ctx.enter_context(tc.tile_pool(name="const", bufs=1))

            ones_col = const.tile([128, 2, 16], fp8, name="ones_col")
            nc.vector.memset(ones_col, 1.0)
            ones_row = const.tile([1, 128], f32, name="ones_row")
            nc.vector.memset(ones_row, 1.0)

            # persistent state
            xT = [
                [
                    const.tile([128, RH], bf16, name=f"xT{dc}{hf}")
                    for hf in range(2)
                ]
                for dc in range(2)
            ]
            eP = [const.tile([128, 2, RH], fp8, name=f"eP{i}") for i in range(NCP)]
            hP = [const.tile([128, 2, D], fp8, name=f"hP{i}") for i in range(NCP)]
            hP2 = [const.tile([128, 2, D], fp8, name=f"hQ{i}") for i in range(NCP)]
            rinvB = const.tile([128, RH], f32, name="rinvB")

            wb_s = const.tile([128, WBCOLS], bf16, name="wb_s")
            fb_s = const.tile([128, 8], f32, name="fb_s")
            kT = const.tile([128, N], bf16, name="kT")
            qT = const.tile([128, RH], bf16, name="qT")

            def wq_sl(dc):
                return wb_s[:, WQ0 + dc * 128 : WQ0 + (dc + 1) * 128]

            def wk_sl(dc):
                return wb_s[:, WK0 + dc * 128 : WK0 + (dc + 1) * 128]

            def gw_sl(it, dc):
                o = GW0 + (it * 2 + dc) * 256
                return wb_s[:, o : o + 256]

            def agg_sl(dc):
                o = AGG0 + dc * 256
                return wb_s[:, o : o + 256]

            wqb_s = fb_s[:, 0:1]
            wkb_s = fb_s[:, 1:2]

            def gb_sl(it, dc):
                return fb_s[:, 2 + it * 2 + dc : 3 + it * 2 + dc]

            m0_s = fb_s[:, 6:7]
            m1_s = fb_s[:, 7:8]

            # weight + bias loads on the gpsimd DMA queue; nodes on sync
            nc.gpsimd.dma_start(out=fb_s, in_=fb[:, :])
            nc.gpsimd.dma_start(out=wb_s, in_=wb[:, :])

            # phase pools
            ps_u = ctx.enter_context(tc.tile_pool(name="ps_u", bufs=2, space="PSUM"))
            ps_h = ctx.enter_context(tc.tile_pool(name="ps_h", bufs=1, space="PSUM"))
            racc = ctx.enter_context(tc.tile_pool(name="racc", bufs=2))
            upd = ctx.enter_context(tc.tile_pool(name="upd", bufs=4))
            dram = ctx.enter_context(tc.tile_pool(name="dram", bufs=1, space="DRAM"))
            cct = ctx.enter_context(tc.tile_pool(name="cct", bufs=8))

            cc_in = [
                dram.tile([4 * 128, 2 * D], fp8, name=f"cc_in{g}") for g in range(2)
            ]
            cc_out = [
                dram.tile([8 * 128, 2 * D], fp8, name=f"cc_out{g}") for g in range(2)
            ]

            def load_quarter(q):
                # quarters 0/1 on sync, 2/3 on gpsimd (behind the small
                # weight loads) so the two DMA queues run in parallel
                hf, base = (q // 2, (q % 2) * 1024)
                eng = nc.sync if q < 2 else nc.gpsimd
                for dc in range(2):
                    eng.dma_start(
                        out=xT[dc][hf][:, base : base + 1024],
                        in_=nodes[
                            dc * 128 : (dc + 1) * 128, q * 1024 : (q + 1) * 1024
                        ],
                    )

            def kq_gen(ps_k, wsl, bias_s, dst, hf, col):
                ps = ps_k.tile([128, RT], f32, name="psk", tag="psk")
                for dc in range(2):
                    nc.tensor.matmul(
                        ps,
                        wsl(dc),
                        xT[dc][hf][:, col : col + RT],
                        start=(dc == 0),
                        stop=(dc == 1),
                    )
                dcol = hf * RH + col
                nc.vector.tensor_scalar_add(
                    out=dst[:, dcol : dcol + RT], in0=ps, scalar1=bias_s
                )

            def sc(mt, ncx):
                # one neighbor chunk of transposed scores for mega-rowtile mt,
                # exp'ed into eP on the Scalar engine
                ps = ps_sc.tile([128, 2 * RT], f32, name="pss", tag="pss")
                for j in range(2):
                    nc.tensor.matmul(
                        ps[:, j * RT : (j + 1) * RT],
                        kT[:, ncx * 128 : (ncx + 1) * 128],
                        qT[:, (2 * mt + j) * RT : (2 * mt + j + 1) * RT],
                        start=True,
                        stop=True,
                    )
                nc.scalar.activation(
                    out=eP[ncx // 2][:, ncx % 2, 2 * mt * RT : (2 * mt + 2) * RT],
                    in_=ps,
                    func=AF.Exp,
                    scale=SCALE,
                )

            def h_chunk(it, ncx):
                hf, col = (0, ncx * 128) if ncx < NCP else (1, (ncx - NCP) * 128)
                ps = ps_h.tile([128, D], f32, name="psh", tag="psh")
                for dc in range(2):
                    nc.tensor.matmul(
                        ps,
                        xT[dc][hf][:, col : col + 128],
                        gw_sl(it, dc),
                        start=(dc == 0),
                        stop=(dc == 1),
                    )
                hdst = hP if it == 0 else hP2
                nc.vector.tensor_copy(out=hdst[ncx // 2][:, ncx % 2, :], in_=ps)

            def r_alloc():
                return ps_r.tile([1, RT], f32, name="psrow", tag="psr")

            def r_step(ps_row, rt, cp, start, stop):
                nc.tensor.matmul(
                    ps_row,
                    ones_col[:, :, 0:1],
                    eP[cp][:, :, rt * RT : (rt + 1) * RT],
                    start=start,
                    stop=stop,
                    perf_mode=mybir.MatmulPerfMode.DoubleRow,
                )

            def r_fin(ps_row, rt):
                # broadcast R across partitions on the PE, then 1/x on DVE
                # (the custom-DVE fast reciprocal doesn't codegen in this
                # toolchain)
                rrow = racc.tile([1, RT], f32, name="rrow", tag="rrow")
                nc.vector.tensor_copy(out=rrow, in_=ps_row)
                ps_b = ps_r.tile([128, RT], f32, name="psb", tag="psr")
                nc.tensor.matmul(ps_b, ones_row, rrow, start=True, stop=True)
                nc.vector.reciprocal(
                    out=rinvB[:, rt * RT : (rt + 1) * RT], in_=ps_b
                )

            def agg_alloc():
                return [
                    ps_u.tile([128, RT], f32, name=f"pu{dc}", tag="pu")
                    for dc in range(2)
                ]

            def agg_step(pu, it, rt, cp, start, stop):
                hx = hP if it == 0 else hP2
                for dc in range(2):
                    nc.tensor.matmul(
                        pu[dc],
                        hx[cp][:, :, dc * 128 : (dc + 1) * 128],
                        eP[cp][:, :, rt * RT : (rt + 1) * RT],
                        start=start,
                        stop=stop,
                        perf_mode=mybir.MatmulPerfMode.DoubleRow,
                    )

            def upd_fin(pu, it, rt):
                # x += relu(agg/R + b): mul, fused bias+relu, residual add (DVE)
                for dc in range(2):
                    t = upd.tile([128, RT], f32, name="updt", tag="updt")
                    nc.vector.tensor_mul(
                        t, pu[dc], rinvB[:, rt * RT : (rt + 1) * RT]
                    )
                    nc.vector.tensor_scalar(
                        out=t,
                        in0=t,
                        scalar1=gb_sl(it, dc),
                        scalar2=0.0,
                        op0=ALU.add,
                        op1=ALU.max,
                    )
                    nc.vector.tensor_add(
                        out=xT[dc][0][:, rt * RT : (rt + 1) * RT],
                        in0=xT[dc][0][:, rt * RT : (rt + 1) * RT],
                        in1=t,
                    )

            def h2_dma(rt):
                # stage this rowtile's h2 pair-tiles into the exchange buffer
                for i, cp in enumerate((2 * rt, 2 * rt + 1)):
                    nc.sync.dma_start(
                        out=cc_in[rt // 2][
                            ((rt % 2) * 2 + i) * 128 : ((rt % 2) * 2 + i + 1) * 128,
                            :,
                        ],
                        in_=hP2[cp][:, :, :].rearrange("p a b -> p (a b)"),
                    )

            def fire_cc(g):
                nc.gpsimd.collective_compute(
                    "AllGather",
                    mybir.AluOpType.bypass,
                    replica_groups=[[0, 1], [2, 3], [4, 5], [6, 7]],
                    ins=[cc_in[g][:, :].opt()],
                    outs=[cc_out[g][:, :].opt()],
                )

            def combine(g):
                # place partner h2 pair-tiles into hP2[8+4g .. 12+4g];
                # rank-select via the m0/m1 input masks (2 fused DVE ops)
                for i in range(4):
                    t0 = cct.tile([128, 2 * D], fp8, name="t0", tag="cct")
                    t1 = cct.tile([128, 2 * D], fp8, name="t1", tag="cct")
                    nc.sync.dma_start(
                        out=t0, in_=cc_out[g][i * 128 : (i + 1) * 128, :]
                    )
                    nc.sync.dma_start(
                        out=t1, in_=cc_out[g][(4 + i) * 128 : (5 + i) * 128, :]
                    )
                    nc.vector.tensor_scalar_mul(t0, t0, m1_s)
                    nc.vector.scalar_tensor_tensor(
                        out=hP2[8 + 4 * g + i][:, :, :].rearrange("p a b -> p (a b)"),
                        in0=t1,
                        scalar=m0_s,
                        in1=t0,
                        op0=ALU.mult,
                        op1=ALU.add,
                    )

            # ---------------- phase A ----------------
            # loads + all kq gen (own PSUM pool, closed before scores pools
            # open), then chunk-paced: scores(0) / h1 / R(rt0) / agg0(rt0)
            # interleaved against the exp(0) drain
            load_quarter(0)
            load_quarter(1)
            load_quarter(2)
            load_quarter(3)
            with tc.tile_pool(name="ps_k", bufs=3, space="PSUM") as ps_k:
                for q in range(2):
                    base = q * 1024
                    for ct in range(2):
                        kq_gen(ps_k, wk_sl, wkb_s, kT, 0, base + ct * RT)
                        kq_gen(ps_k, wq_sl, wqb_s, qT, 0, base + ct * RT)
                for q in range(2):
                    base = q * 1024
                    for ct in range(2):
                        kq_gen(ps_k, wk_sl, wkb_s, kT, 1, base + ct * RT)

            p2 = ExitStack()
            ps_r = p2.enter_context(tc.tile_pool(name="ps_r", bufs=2, space="PSUM"))
            p1 = ExitStack()
            ps_sc = p1.enter_context(tc.tile_pool(name="ps_sc", bufs=1, space="PSUM"))

            pr0 = r_alloc()
            pu0 = agg_alloc()

            def ab_tail(c):
                # interleaved consumers trailing the exp stream by 4 chunks
                if c >= 4 and c % 2 == 0:
                    cp = (c - 4) // 2
                    r_step(pr0, 0, cp, start=(cp == 0), stop=False)
                    agg_step(pu0, 0, 0, cp, start=(cp == 0), stop=False)

            for c in range(32):
                sc(0, c)
                h_chunk(0, c)
                ab_tail(c)
            # drain rt0 pair-steps cp=14,15 and finish
            for cp in (14, 15):
                r_step(pr0, 0, cp, start=False, stop=(cp == 15))
                agg_step(pu0, 0, 0, cp, start=False, stop=(cp == 15))
            r_fin(pr0, 0)
            upd_fin(pu0, 0, 0)

            # ---------------- phase B ----------------
            # scores(1) feeds exp(1) continuously (PE work here sized to the
            # Scalar pace); rt1 blocks run on exp(0), rt2 is chunk-paced
            # against the exp(1) drain; h2 + the first AllGather fire as
            # soon as x1 of rowtiles 0/1 exists
            for c in range(0, 2):
                sc(1, c)
            pr1 = r_alloc()
            for cp in range(NCP):
                r_step(pr1, 1, cp, start=(cp == 0), stop=(cp == NCP - 1))
            r_fin(pr1, 1)
            for c in range(2, 4):
                sc(1, c)
            # h2 of rowtile 0 (x1 updated at the A/B boundary)
            for ncx in range(0, 4):
                h_chunk(1, ncx)
            h2_dma(0)
            pu1 = agg_alloc()
            for c in range(4, 8):
                sc(1, c)
                agg_step(pu1, 0, 1, 2 * (c - 4), start=(c == 4), stop=False)
                agg_step(pu1, 0, 1, 2 * (c - 4) + 1, start=False, stop=False)
            for c in range(8, 12):
                sc(1, c)
                agg_step(pu1, 0, 1, 2 * (c - 4), start=False, stop=False)
                agg_step(
                    pu1, 0, 1, 2 * (c - 4) + 1, start=False,
                    stop=(c == 11),
                )
            upd_fin(pu1, 0, 1)
            for c in range(12, 14):
                sc(1, c)
                h_chunk(1, 4 + 2 * (c - 12))
                h_chunk(1, 4 + 2 * (c - 12) + 1)
            h2_dma(1)
            fire_cc(0)
            for c in range(14, 16):
                sc(1, c)
            # chunk-paced rt2 agg + R(rt2)/R(rt3) against the exp(1) drain
            # (paced pairs 0..7, drained pairs 8..15)
            pr2 = r_alloc()
            pr3 = r_alloc()
            pu2 = agg_alloc()
            for c in range(16, 32):
                sc(1, c)
                if c % 2 == 1:
                    cp = (c - 17) // 2
                    r_step(pr2, 2, cp, start=(cp == 0), stop=False)
                    r_step(pr3, 3, cp, start=(cp == 0), stop=False)
                    agg_step(pu2, 0, 2, cp, start=(cp == 0), stop=False)
            for cp in range(8, NCP):
                r_step(pr2, 2, cp, start=False, stop=(cp == NCP - 1))
                r_step(pr3, 3, cp, start=False, stop=(cp == NCP - 1))
                agg_step(pu2, 0, 2, cp, start=False, stop=(cp == NCP - 1))
            r_fin(pr2, 2)
            upd_fin(pu2, 0, 2)
            r_fin(pr3, 3)
            p1.close()
            p2.close()

            # ---------------- phase C (post-exp) ----------------
            # agg(rt3) on the freshly opened pool (no DVE gate); h2 of
            # rowtiles 2/3 and the start of the iter-2 aggregation cover
            # the DVE update chains; the second exchange fires as early as
            # its dependencies allow
            ps_u2 = ctx.enter_context(tc.tile_pool(name="ps_u2", bufs=2, space="PSUM"))

            def agg_alloc2(pool, nm):
                return [
                    pool.tile([128, RT], f32, name=f"{nm}{dc}", tag="pu")
                    for dc in range(2)
                ]

            pu3 = agg_alloc2(ps_u2, "pw")
            for cp in range(NCP):
                agg_step(pu3, 0, 3, cp, start=(cp == 0), stop=(cp == NCP - 1))
            for ncx in range(8, 12):
                h_chunk(1, ncx)
            h2_dma(2)
            upd_fin(pu3, 0, 3)
            # start iter-2 aggregation of rowtile 0 (local pairs) to keep
            # the PE busy while the DVE finishes upd(rt3)
            pair_order = list(range(8)) + [8, 9, 10, 11] + [12, 13, 14, 15]
            pv0 = agg_alloc2(ps_u, "pv")
            for cp in range(8):
                agg_step(pv0, 1, 0, cp, start=(cp == 0), stop=False)
            for ncx in range(12, 16):
                h_chunk(1, ncx)
            h2_dma(3)
            fire_cc(1)
            combine(0)

            # ---------------- phase D ----------------
            pso = ctx.enter_context(tc.tile_pool(name="pso", bufs=3, space="PSUM"))
            ost = ctx.enter_context(tc.tile_pool(name="ost", bufs=4))
            combine(1)

            def agg1(rt, pu=None, skip=0):
                # alternate between two PSUM pools so consecutive rowtile
                # accumulations overlap
                if pu is None:
                    pu = agg_alloc2(ps_u if rt % 2 == 0 else ps_u2, "pv")
                for i, cp in enumerate(pair_order):
                    if i < skip:
                        continue
                    agg_step(pu, 1, rt, cp, start=(i == 0), stop=(i == NCP - 1))
                return pu

            def out_chunk(rc, qi):
                ps = pso.tile([128, D], f32, name="pso", tag="pso")
                for dc in range(2):
                    nc.tensor.matmul(
                        ps,
                        xT[dc][0][:, rc * 128 : (rc + 1) * 128],
                        agg_sl(dc),
                        start=(dc == 0),
                        stop=(dc == 1),
                    )
                ot = ost.tile([128, D], f32, name="ot", tag="ot")
                nc.vector.tensor_copy(out=ot, in_=ps)
                eng = nc.sync if qi % 2 == 0 else nc.scalar
                eng.dma_start(out=part[rc * 128 : (rc + 1) * 128, :], in_=ot)

            pus = [agg1(0, pu=pv0, skip=8), agg1(1)]
            upd_fin(pus[0], 1, 0)
            for rt in range(NRT):
                if rt + 2 < NRT:
                    pus.append(agg1(rt + 2))
                if rt + 1 < NRT:
                    upd_fin(pus[rt + 1], 1, rt + 1)
                for rc in range(4 * rt, 4 * rt + 4):
                    out_chunk(rc, rc)

    _split_excess_waits(nc, mybir)
    return nc


def _get_nc():
    if "nc" not in _CACHE:
        _CACHE["nc"] = _build()
    return _CACHE["nc"]


def _in_maps(inputs):
    import ml_dtypes

    bf16 = ml_dtypes.bfloat16

    ne = np.asarray(inputs["nodes_embed"], dtype=np.float32)
    wq_w = np.asarray(inputs["WQ_w"], dtype=np.float32)
    wq_b = np.asarray(inputs["WQ_b"], dtype=np.float32)
    wk_w = np.asarray(inputs["WK_w"], dtype=np.float32)
    wk_b = np.asarray(inputs["WK_b"], dtype=np.float32)
    gcn_w = np.asarray(inputs["gcn_W"], dtype=np.float32)
    gcn_b = np.asarray(inputs["gcn_b"], dtype=np.float32)
    agg_w = np.asarray(inputs["agg_W"], dtype=np.float32)

    maps = []
    for c in range(8):
        b, h, rh = c // 4, (c // 2) % 2, c % 2
        if rh == 0:
            nodes = ne[b]
        else:
            nodes = np.concatenate([ne[b, RH:], ne[b, :RH]], axis=0)
        nodes = np.ascontiguousarray(nodes.T).astype(bf16)  # [D, N], x^T

        wq_h = wq_w[:, h * DK : (h + 1) * DK]
        wk_h = wk_w[:, h * DK : (h + 1) * DK]
        agg_h = agg_w[h * D : (h + 1) * D, :]
        wbm = np.zeros((128, WBCOLS), np.float32)
        wbm[:, WQ0 : WQ0 + 128] = wq_h[0:128, :]
        wbm[:, WQ0 + 128 : WQ0 + 256] = wq_h[128:256, :]
        wbm[:, WK0 : WK0 + 128] = wk_h[0:128, :]
        wbm[:, WK0 + 128 : WK0 + 256] = wk_h[128:256, :]
        for it in range(ITERS):
            for dc in range(2):
                o = GW0 + (it * 2 + dc) * 256
                wbm[:, o : o + 256] = gcn_w[it, dc * 128 : (dc + 1) * 128, :]
        for dc in range(2):
            o = AGG0 + dc * 256
            wbm[:, o : o + 256] = agg_h[dc * 128 : (dc + 1) * 128, :]

        fbm = np.zeros((128, 8), np.float32)
        fbm[:, 0] = wq_b[h * DK : (h + 1) * DK]
        fbm[:, 1] = wk_b[h * DK : (h + 1) * DK]
        for it in range(ITERS):
            for dc in range(2):
                fbm[:, 2 + it * 2 + dc] = gcn_b[it, dc * 128 : (dc + 1) * 128]
        fbm[:, 6] = 1.0 if rh == 0 else 0.0
        fbm[:, 7] = 0.0 if rh == 0 else 1.0

        maps.append(
            {
                "nodes": nodes,
                "wb": np.ascontiguousarray(wbm.astype(bf16)),
                "fb": np.ascontiguousarray(fbm),
            }
        )
    return maps


def kernel(trace=False, tmpdir=None, **inputs):
    from concourse.bass_utils import run_bass_kernel_spmd

    nc = _get_nc()
    maps = _in_maps(inputs)
    kw = {}
    if trace:
        kw = dict(trace=True, tmpdir=tmpdir)
    res = run_bass_kernel_spmd(nc, maps, core_ids=list(range(8)), **kw)

    agg_b = np.asarray(inputs["agg_b"], dtype=np.float32)
    out = np.zeros((B, N, D), np.float32)
    for b in range(B):
        for rh in range(2):
            rows = slice(rh * RH, (rh + 1) * RH)
            out[b, rows, :] = (
                res.results[4 * b + 0 * 2 + rh]["part"]
                + res.results[4 * b + 1 * 2 + rh]["part"]
                + agg_b
            )
    if trace:
        return out, res
    return out


# revision 26
# speedup vs baseline: 1.2271x; 1.2271x over previous
"""Trainium2 Bass kernel for nn_AttentionGCNLayer (B=2, N=4096, D=256, H=2, ITERS=2).

Sharding: 8 cores = (b in 2) x (h in 2) x (row-half in 2). Each core handles one
(batch, head) pair and one half (2048) of the attention rows, with a pairwise
AllGather of the updated node features between the two GCN iterations.

Schedule (v2): chunk-paced pipeline. The Scalar engine's exp stream (64
activations of [128,1024], ~73us) is the second wall after the PE (~114us);
the program interleaves PE work (scores, h-gen, R rowsums, agg accumulation)
at neighbor-chunk granularity so the PE is never idle waiting on exp:

  A:  kq gen + scores(mega-tile 0) exp-paced, h1 + R(rt0) + agg(rt0)
      partial accumulation interleaved per chunk-pair.
  B:  scores(mega-tile 1) feeding exp(1), R(rt1)/agg(rt1) blocks, then
      chunk-paced R(rt2)/agg(rt2) against the exp(1) drain; h2 + AllGather
      of updated features fire per row-pair as soon as x1 is ready.
  D:  iter-2 aggregation (pair order: cc0-half first, local, cc1-half last
      to match AllGather arrival), output projection + DMA interleaved.

Layout: x kept transposed (x^T [D on 2x128 partitions, N free]) in local row
order; scores computed transposed (E^T = exp(q k^T)^T, neighbors on
partitions) feeding the aggregation matmuls directly; E and h in fp8 with
DoubleRow matmuls; weights arrive pre-cast to bf16 from the host in a single
packed blob (no on-device staging casts). Softmax normalizer R = rowsum(E)
via DoubleRow ones-matmuls; 1/R via the fast DVE reciprocal. Scalar engine
runs exp only; copies/relu/bias live on the Vector engine.
"""

import sys

if "/opt/trn_rl_repo" not in sys.path:
    sys.path.insert(0, "/opt/trn_rl_repo")

import numpy as np

B, N, D, H, ITERS = 2, 4096, 256, 2, 2
DK = D // H                      # 128
RH = N // 2                      # 2048 rows per core
NCH = N // 128                   # 32 neighbor chunks
NCP = NCH // 2                   # 16 neighbor chunk-pairs
RT = 512                         # row tile (one PSUM bank of fp32)
NRT = RH // RT                   # 4 row tiles per core
SCALE = 1.0 / float(np.sqrt(np.float32(DK)))

# packed bf16 weight blob column offsets
WQ0, WK0, GW0, AGG0 = 0, 256, 512, 1536
WBCOLS = 2048

_CACHE = {}


def _seq_engines(mybir):
    return {
        mybir.EngineType.PE,
        mybir.EngineType.Activation,
        mybir.EngineType.Pool,
        mybir.EngineType.DVE,
        mybir.EngineType.SP,
    }


def _split_excess_waits(nc, mybir, max_waits=1):
    """This container's walrus accepts at most one sync-wait per engine
    instruction; hoist extra waits onto preceding NoOps on the same engine."""
    seq = _seq_engines(mybir)
    n_new = 0
    for f in nc.m.functions:
        for blk in f.blocks:
            if not any(
                inst.sync_info is not None
                and inst.sync_info.on_wait
                and len(inst.sync_info.on_wait) > max_waits
                and inst.engine in seq
                for inst in blk.instructions
            ):
                continue
            out = []
            for inst in blk.instructions:
                si = inst.sync_info
                if (
                    si is not None
                    and si.on_wait
                    and len(si.on_wait) > max_waits
                    and inst.engine in seq
                ):
                    waits = list(si.on_wait)
                    keep, extra = waits[:max_waits], waits[max_waits:]
                    while extra:
                        chunk, extra = extra[:max_waits], extra[max_waits:]
                        out.append(
                            mybir.InstNoOp(
                                name=f"{inst.name}-ws{n_new}",
                                sync_info=mybir.SyncInfo(on_wait=chunk, on_update=[]),
                                bass_nofuse=True,
                                engine=inst.engine,
                            )
                        )
                        n_new += 1
                    inst.sync_info = mybir.SyncInfo(
                        on_wait=keep, on_update=list(si.on_update)
                    )
                out.append(inst)
            blk.instructions = out
    return n_new


def _build():
    import concourse.bass as bass
    import concourse.mybir as mybir
    import concourse.tile as tile

    f32 = mybir.dt.float32
    bf16 = mybir.dt.bfloat16
    fp8 = mybir.dt.float8e4
    AF = mybir.ActivationFunctionType
    ALU = mybir.AluOpType

    nc = bass.Bass("TRN2", num_devices=8)

    nodes = nc.dram_tensor("nodes", [D, N], bf16, kind="ExternalInput")
    wb = nc.dram_tensor("wb", [128, WBCOLS], bf16, kind="ExternalInput")
    fb = nc.dram_tensor("fb", [128, 8], f32, kind="ExternalInput")
    part = nc.dram_tensor("part", [RH, D], f32, kind="ExternalOutput")

    with tile.TileContext(nc) as tc:
        from contextlib import ExitStack

        with ExitStack() as ctx:
            const = ctx.enter_context(tc.tile_pool(name="const", bufs=1))

            ones_col = const.tile([128, 2, 16], fp8, name="ones_col")
            nc.vector.memset(ones_col, 1.0)
            ones_row = const.tile([1, 128], f32, name="ones_row")
            nc.vector.memset(ones_row, 1.0)

            # persistent state
            xT = [
                [
                    const.tile([128, RH], bf16, name=f"xT{dc}{hf}")
                    for hf in range(2)
                ]
                for dc in range(2)
            ]
            eP = [const.tile([128, 2, RH], fp8, name=f"eP{i}") for i in range(NCP)]
            hP = [const.tile([128, 2, D], fp8, name=f"hP{i}") for i in range(NCP)]
            hP2 = [const.tile([128, 2, D], fp8, name=f"hQ{i}") for i in range(NCP)]
            rinvB = const.tile([128, RH], f32, name="rinvB")

            wb_s = const.tile([128, WBCOLS], bf16, name="wb_s")
            fb_s = const.tile([128, 8], f32, name="fb_s")
            kT = const.tile([128, N], bf16, name="kT")
            qT = const.tile([128, RH], bf16, name="qT")

            def wq_sl(dc):
                return wb_s[:, WQ0 + dc * 128 : WQ0 + (dc + 1) * 128]

            def wk_sl(dc):
                return wb_s[:, WK0 + dc * 128 : WK0 + (dc + 1) * 128]

            def gw_sl(it, dc):
                o = GW0 + (it * 2 + dc) * 256
                return wb_s[:, o : o + 256]

            def agg_sl(dc):
                o = AGG0 + dc * 256
                return wb_s[:, o : o + 256]

            wqb_s = fb_s[:, 0:1]
            wkb_s = fb_s[:, 1:2]

            def gb_sl(it, dc):
                return fb_s[:, 2 + it * 2 + dc : 3 + it * 2 + dc]

            m0_s = fb_s[:, 6:7]
            m1_s = fb_s[:, 7:8]

            # weight + bias loads on the gpsimd DMA queue; nodes on sync
            nc.gpsimd.dma_start(out=fb_s, in_=fb[:, :])
            nc.gpsimd.dma_start(out=wb_s, in_=wb[:, :])

            # phase pools
            ps_u = ctx.enter_context(tc.tile_pool(name="ps_u", bufs=2, space="PSUM"))
            ps_h = ctx.enter_context(tc.tile_pool(name="ps_h", bufs=1, space="PSUM"))
            racc = ctx.enter_context(tc.tile_pool(name="racc", bufs=2))
            upd = ctx.enter_context(tc.tile_pool(name="upd", bufs=4))
            dram = ctx.enter_context(tc.tile_pool(name="dram", bufs=1, space="DRAM"))
            cct = ctx.enter_context(tc.tile_pool(name="cct", bufs=8))

            cc_in = [
                dram.tile([4 * 128, 2 * D], fp8, name=f"cc_in{g}") for g in range(2)
            ]
            cc_out = [
                dram.tile([8 * 128, 2 * D], fp8, name=f"cc_out{g}") for g in range(2)
            ]

            def load_quarter(q):
                # quarters 0/1 on sync, 2/3 on gpsimd (behind the small
                # weight loads) so the two DMA queues run in parallel
                hf, base = (q // 2, (q % 2) * 1024)
                eng = nc.sync if q < 2 else nc.gpsimd
                for dc in range(2):
                    eng.dma_start(
                        out=xT[dc][hf][:, base : base + 1024],
                        in_=nodes[
                            dc * 128 : (dc + 1) * 128, q * 1024 : (q + 1) * 1024
                        ],
                    )

            def kq_gen(ps_k, wsl, bias_s, dst, hf, col):
                ps = ps_k.tile([128, RT], f32, name="psk", tag="psk")
                for dc in range(2):
                    nc.tensor.matmul(
                        ps,
                        wsl(dc),
                        xT[dc][hf][:, col : col + RT],
                        start=(dc == 0),
                        stop=(dc == 1),
                    )
                dcol = hf * RH + col
                nc.vector.tensor_scalar_add(
                    out=dst[:, dcol : dcol + RT], in0=ps, scalar1=bias_s
                )

            def sc(mt, ncx):
                # one neighbor chunk of transposed scores for mega-rowtile mt,
                # exp'ed into eP on the Scalar engine
                ps = ps_sc.tile([128, 2 * RT], f32, name="pss", tag="pss")
                for j in range(2):
                    nc.tensor.matmul(
                        ps[:, j * RT : (j + 1) * RT],
                        kT[:, ncx * 128 : (ncx + 1) * 128],
                        qT[:, (2 * mt + j) * RT : (2 * mt + j + 1) * RT],
                        start=True,
                        stop=True,
                    )
                nc.scalar.activation(
                    out=eP[ncx // 2][:, ncx % 2, 2 * mt * RT : (2 * mt + 2) * RT],
                    in_=ps,
                    func=AF.Exp,
                    scale=SCALE,
                )

            def h_chunk(it, ncx):
                hf, col = (0, ncx * 128) if ncx < NCP else (1, (ncx - NCP) * 128)
                ps = ps_h.tile([128, D], f32, name="psh", tag="psh")
                for dc in range(2):
                    nc.tensor.matmul(
                        ps,
                        xT[dc][hf][:, col : col + 128],
                        gw_sl(it, dc),
                        start=(dc == 0),
                        stop=(dc == 1),
                    )
                hdst = hP if it == 0 else hP2
                nc.vector.tensor_copy(out=hdst[ncx // 2][:, ncx % 2, :], in_=ps)

            def r_alloc():
                return ps_r.tile([1, RT], f32, name="psrow", tag="psr")

            def r_step(ps_row, rt, cp, start, stop):
                nc.tensor.matmul(
                    ps_row,
                    ones_col[:, :, 0:1],
                    eP[cp][:, :, rt * RT : (rt + 1) * RT],
                    start=start,
                    stop=stop,
                    perf_mode=mybir.MatmulPerfMode.DoubleRow,
                )

            def r_fin(ps_row, rt):
                # broadcast R across partitions on the PE, then 1/x on DVE
                # (the custom-DVE fast reciprocal doesn't codegen in this
                # toolchain)
                rrow = racc.tile([1, RT], f32, name="rrow", tag="rrow")
                nc.vector.tensor_copy(out=rrow, in_=ps_row)
                ps_b = ps_r.tile([128, RT], f32, name="psb", tag="psr")
                nc.tensor.matmul(ps_b, ones_row, rrow, start=True, stop=True)
                nc.vector.reciprocal(
                    out=rinvB[:, rt * RT : (rt + 1) * RT], in_=ps_b
                )

            def agg_alloc():
                return [
                    ps_u.tile([128, RT], f32, name=f"pu{dc}", tag="pu")
                    for dc in range(2)
                ]

            def agg_step(pu, it, rt, cp, start, stop):
                hx = hP if it == 0 else hP2
                for dc in range(2):
                    nc.tensor.matmul(
                        pu[dc],
                        hx[cp][:, :, dc * 128 : (dc + 1) * 128],
                        eP[cp][:, :, rt * RT : (rt + 1) * RT],
                        start=start,
                        stop=stop,
                        perf_mode=mybir.MatmulPerfMode.DoubleRow,
                    )

            def upd_fin(pu, it, rt):
                # x += relu(agg/R + b): mul, fused bias+relu, residual add (DVE)
                for dc in range(2):
                    t = upd.tile([128, RT], f32, name="updt", tag="updt")
                    nc.vector.tensor_mul(
                        t, pu[dc], rinvB[:, rt * RT : (rt + 1) * RT]
                    )
                    nc.vector.tensor_scalar(
                        out=t,
                        in0=t,
                        scalar1=gb_sl(it, dc),
                        scalar2=0.0,
                        op0=ALU.add,
                        op1=ALU.max,
                    )
                    nc.vector.tensor_add(
                        out=xT[dc][0][:, rt * RT : (rt + 1) * RT],
                        in0=xT[dc][0][:, rt * RT : (rt + 1) * RT],
                        in1=t,
                    )

            def h2_dma(rt):
                # stage this rowtile's h2 pair-tiles into the exchange buffer
                for i, cp in enumerate((2 * rt, 2 * rt + 1)):
                    nc.sync.dma_start(
                        out=cc_in[rt // 2][
                            ((rt % 2) * 2 + i) * 128 : ((rt % 2) * 2 + i + 1) * 128,
                            :,
                        ],
                        in_=hP2[cp][:, :, :].rearrange("p a b -> p (a b)"),
                    )

            def fire_cc(g):
                nc.gpsimd.collective_compute(
                    "AllGather",
                    mybir.AluOpType.bypass,
                    replica_groups=[[0, 1], [2, 3], [4, 5], [6, 7]],
                    ins=[cc_in[g][:, :].opt()],
                    outs=[cc_out[g][:, :].opt()],
                )

            def combine(g):
                # place partner h2 pair-tiles into hP2[8+4g .. 12+4g];
                # rank-select via the m0/m1 input masks (2 fused DVE ops)
                for i in range(4):
                    t0 = cct.tile([128, 2 * D], fp8, name="t0", tag="cct")
                    t1 = cct.tile([128, 2 * D], fp8, name="t1", tag="cct")
                    nc.sync.dma_start(
                        out=t0, in_=cc_out[g][i * 128 : (i + 1) * 128, :]
                    )
                    nc.sync.dma_start(
                        out=t1, in_=cc_out[g][(4 + i) * 128 : (5 + i) * 128, :]
                    )
                    nc.vector.tensor_scalar_mul(t0, t0, m1_s)
                    nc.vector.scalar_tensor_tensor(
                        out=hP2[8 + 4 * g + i][:, :, :].rearrange("p a b -> p (a b)"),
                        in0=t1,
                        scalar=m0_s,
                        in1=t0,
                        op0=ALU.mult,
                        op1=ALU.add,
                    )

            # ---------------- phase A ----------------
            # loads + all kq gen (own PSUM pool, closed before scores pools
            # open), then chunk-paced: scores(0) / h1 / R(rt0) / agg0(rt0)
            # interleaved against the exp(0) drain
            load_quarter(0)
            load_quarter(1)
            load_quarter(2)
            load_quarter(3)
            with tc.tile_pool(name="ps_k", bufs=3, space="PSUM") as ps_k:
                for q in range(2):
                    base = q * 1024
                    for ct in range(2):
                        kq_gen(ps_k, wk_sl, wkb_s, kT, 0, base + ct * RT)
                        kq_gen(ps_k, wq_sl, wqb_s, qT, 0, base + ct * RT)
                for q in range(2):
                    base = q * 1024
                    for ct in range(2):
                        kq_gen(ps_k, wk_sl, wkb_s, kT, 1, base + ct * RT)

            p2 = ExitStack()
            ps_r = p2.enter_context(tc.tile_pool(name="ps_r", bufs=1, space="PSUM"))
            p1 = ExitStack()
            ps_sc = p1.enter_context(tc.tile_pool(name="ps_sc", bufs=2, space="PSUM"))

            pr0 = r_alloc()
            pu0 = agg_alloc()

            def ab_tail(c):
                # interleaved consumers trailing the exp stream by 4 chunks
                if c >= 4 and c % 2 == 0:
                    cp = (c - 4) // 2
                    r_step(pr0, 0, cp, start=(cp == 0), stop=False)
                    agg_step(pu0, 0, 0, cp, start=(cp == 0), stop=False)

            for c in range(32):
                sc(0, c)
                h_chunk(0, c)
                ab_tail(c)
            # drain rt0 pair-steps cp=14,15 and finish
            for cp in (14, 15):
                r_step(pr0, 0, cp, start=False, stop=(cp == 15))
                agg_step(pu0, 0, 0, cp, start=False, stop=(cp == 15))
            r_fin(pr0, 0)
            upd_fin(pu0, 0, 0)

            # ---------------- phase B ----------------
            # scores(1) feeds exp(1) continuously (PE work here sized to the
            # Scalar pace); rt1 blocks run on exp(0), rt2 is chunk-paced
            # against the exp(1) drain; h2 + the first AllGather fire as
            # soon as x1 of rowtiles 0/1 exists
            for c in range(0, 2):
                sc(1, c)
            pr1 = r_alloc()
            for cp in range(NCP):
                r_step(pr1, 1, cp, start=(cp == 0), stop=(cp == NCP - 1))
            r_fin(pr1, 1)
            for c in range(2, 4):
                sc(1, c)
            # h2 of rowtile 0 (x1 updated at the A/B boundary)
            for ncx in range(0, 4):
                h_chunk(1, ncx)
            h2_dma(0)
            pu1 = agg_alloc()
            for c in range(4, 8):
                sc(1, c)
                agg_step(pu1, 0, 1, 2 * (c - 4), start=(c == 4), stop=False)
                agg_step(pu1, 0, 1, 2 * (c - 4) + 1, start=False, stop=False)
            for c in range(8, 12):
                sc(1, c)
                agg_step(pu1, 0, 1, 2 * (c - 4), start=False, stop=False)
                agg_step(
                    pu1, 0, 1, 2 * (c - 4) + 1, start=False,
                    stop=(c == 11),
                )
            upd_fin(pu1, 0, 1)
            for c in range(12, 14):
                sc(1, c)
                h_chunk(1, 4 + 2 * (c - 12))
                h_chunk(1, 4 + 2 * (c - 12) + 1)
            h2_dma(1)
            fire_cc(0)
            for c in range(14, 16):
                sc(1, c)
            # chunk-paced rt2 agg + R(rt2) against the exp(1) drain
            # (paced pairs 0..7, drained pairs 8..15)
            pr2 = r_alloc()
            pu2 = agg_alloc()
            for c in range(16, 32):
                sc(1, c)
                if c % 2 == 1:
                    cp = (c - 17) // 2
                    r_step(pr2, 2, cp, start=(cp == 0), stop=False)
                    agg_step(pu2, 0, 2, cp, start=(cp == 0), stop=False)
            for cp in range(8, NCP):
                r_step(pr2, 2, cp, start=False, stop=(cp == NCP - 1))
                agg_step(pu2, 0, 2, cp, start=False, stop=(cp == NCP - 1))
            r_fin(pr2, 2)
            upd_fin(pu2, 0, 2)
            p1.close()
            p2.close()

            # ---------------- phase C (post-exp) ----------------
            # agg(rt3) + R(rt3) on freshly opened pools (no DVE gates); h2
            # of rowtiles 2/3 and the start of the iter-2 aggregation cover
            # the DVE update chains; the second exchange fires as early as
            # its dependencies allow
            ps_u2 = ctx.enter_context(tc.tile_pool(name="ps_u2", bufs=2, space="PSUM"))

            def agg_alloc2(pool, nm):
                return [
                    pool.tile([128, RT], f32, name=f"{nm}{dc}", tag="pu")
                    for dc in range(2)
                ]

            pu3 = agg_alloc2(ps_u2, "pw")
            with tc.tile_pool(name="ps_c", bufs=1, space="PSUM") as ps_c:
                pr3 = ps_c.tile([1, RT], f32, name="psrow3", tag="psc")
                for cp in range(NCP):
                    agg_step(pu3, 0, 3, cp, start=(cp == 0), stop=(cp == NCP - 1))
                    r_step(pr3, 3, cp, start=(cp == 0), stop=(cp == NCP - 1))
                rrow3 = racc.tile([1, RT], f32, name="rrow", tag="rrow")
                nc.vector.tensor_copy(out=rrow3, in_=pr3)
                psb3 = ps_c.tile([128, RT], f32, name="psb3", tag="psc")
                nc.tensor.matmul(psb3, ones_row, rrow3, start=True, stop=True)
                nc.vector.reciprocal(out=rinvB[:, 3 * RT : 4 * RT], in_=psb3)
                for ncx in range(8, 12):
                    h_chunk(1, ncx)
                h2_dma(2)
                upd_fin(pu3, 0, 3)
                # start iter-2 aggregation (pass 1: local pairs of rowtile
                # 0) to keep the PE busy while the DVE finishes upd(rt3)
                pv0 = agg_alloc2(ps_u, "pv")
                for cp in range(8):
                    agg_step(pv0, 1, 0, cp, start=(cp == 0), stop=False)
                for ncx in range(12, 16):
                    h_chunk(1, ncx)
                h2_dma(3)
                fire_cc(1)
                combine(0)

            # ---------------- phase D ----------------
            # two-pass iter-2 aggregation: pass 1 = local + cc0 pairs into
            # an SBUF partial (already scaled by 1/R); pass 2 = cc1 pairs
            # once the second AllGather lands. No rowtile ever waits on
            # cc(1) with PSUM held.
            pso = ctx.enter_context(tc.tile_pool(name="pso", bufs=3, space="PSUM"))
            ost = ctx.enter_context(tc.tile_pool(name="ost", bufs=4))
            pp = ctx.enter_context(tc.tile_pool(name="pp", bufs=1))
            combine(1)

            def pass1(rt, pu=None, skip=0):
                if pu is None:
                    pu = agg_alloc2(ps_u if rt % 2 == 0 else ps_u2, "pv")
                for i in range(skip, 12):
                    agg_step(pu, 1, rt, i, start=(i == 0), stop=(i == 11))
                parts = []
                for dc in range(2):
                    pt = pp.tile([128, RT], f32, name=f"pp{rt}{dc}", tag=f"pp{rt}{dc}")
                    nc.vector.tensor_mul(
                        pt, pu[dc], rinvB[:, rt * RT : (rt + 1) * RT]
                    )
                    parts.append(pt)
                return parts

            def pass2(rt, parts):
                pu = agg_alloc2(ps_u if rt % 2 == 0 else ps_u2, "pv")
                for i, cp in enumerate(range(12, 16)):
                    agg_step(pu, 1, rt, cp, start=(i == 0), stop=(i == 3))
                for dc in range(2):
                    t = upd.tile([128, RT], f32, name="updt", tag="updt")
                    nc.vector.tensor_mul(
                        t, pu[dc], rinvB[:, rt * RT : (rt + 1) * RT]
                    )
                    nc.vector.tensor_add(out=t, in0=t, in1=parts[dc])
                    nc.vector.tensor_scalar(
                        out=t,
                        in0=t,
                        scalar1=gb_sl(1, dc),
                        scalar2=0.0,
                        op0=ALU.add,
                        op1=ALU.max,
                    )
                    nc.vector.tensor_add(
                        out=xT[dc][0][:, rt * RT : (rt + 1) * RT],
                        in0=xT[dc][0][:, rt * RT : (rt + 1) * RT],
                        in1=t,
                    )

            def out_chunk(rc, qi):
                ps = pso.tile([128, D], f32, name="pso", tag="pso")
                for dc in range(2):
                    nc.tensor.matmul(
                        ps,
                        xT[dc][0][:, rc * 128 : (rc + 1) * 128],
                        agg_sl(dc),
                        start=(dc == 0),
                        stop=(dc == 1),
                    )
                ot = ost.tile([128, D], f32, name="ot", tag="ot")
                nc.vector.tensor_copy(out=ot, in_=ps)
                eng = nc.sync if qi % 2 == 0 else nc.scalar
                eng.dma_start(out=part[rc * 128 : (rc + 1) * 128, :], in_=ot)

            parts = [pass1(0, pu=pv0, skip=8)]
            for rt in range(1, NRT):
                parts.append(pass1(rt))
            # skew the output projection one rowtile behind pass 2 so the
            # PE never waits on the DVE update chains
            for rt in range(NRT):
                pass2(rt, parts[rt])
                if rt > 0:
                    for rc in range(4 * (rt - 1), 4 * rt):
                        out_chunk(rc, rc)
            for rc in range(4 * (NRT - 1), 4 * NRT):
                out_chunk(rc, rc)

    _split_excess_waits(nc, mybir)
    return nc


def _get_nc():
    if "nc" not in _CACHE:
        _CACHE["nc"] = _build()
    return _CACHE["nc"]


def _in_maps(inputs):
    import ml_dtypes

    bf16 = ml_dtypes.bfloat16

    ne = np.asarray(inputs["nodes_embed"], dtype=np.float32)
    wq_w = np.asarray(inputs["WQ_w"], dtype=np.float32)
    wq_b = np.asarray(inputs["WQ_b"], dtype=np.float32)
    wk_w = np.asarray(inputs["WK_w"], dtype=np.float32)
    wk_b = np.asarray(inputs["WK_b"], dtype=np.float32)
    gcn_w = np.asarray(inputs["gcn_W"], dtype=np.float32)
    gcn_b = np.asarray(inputs["gcn_b"], dtype=np.float32)
    agg_w = np.asarray(inputs["agg_W"], dtype=np.float32)

    maps = []
    for c in range(8):
        b, h, rh = c // 4, (c // 2) % 2, c % 2
        if rh == 0:
            nodes = ne[b]
        else:
            nodes = np.concatenate([ne[b, RH:], ne[b, :RH]], axis=0)
        nodes = np.ascontiguousarray(nodes.T).astype(bf16)  # [D, N], x^T

        wq_h = wq_w[:, h * DK : (h + 1) * DK]
        wk_h = wk_w[:, h * DK : (h + 1) * DK]
        agg_h = agg_w[h * D : (h + 1) * D, :]
        wbm = np.zeros((128, WBCOLS), np.float32)
        wbm[:, WQ0 : WQ0 + 128] = wq_h[0:128, :]
        wbm[:, WQ0 + 128 : WQ0 + 256] = wq_h[128:256, :]
        wbm[:, WK0 : WK0 + 128] = wk_h[0:128, :]
        wbm[:, WK0 + 128 : WK0 + 256] = wk_h[128:256, :]
        for it in range(ITERS):
            for dc in range(2):
                o = GW0 + (it * 2 + dc) * 256
                wbm[:, o : o + 256] = gcn_w[it, dc * 128 : (dc + 1) * 128, :]
        for dc in range(2):
            o = AGG0 + dc * 256
            wbm[:, o : o + 256] = agg_h[dc * 128 : (dc + 1) * 128, :]

        fbm = np.zeros((128, 8), np.float32)
        fbm[:, 0] = wq_b[h * DK : (h + 1) * DK]
        fbm[:, 1] = wk_b[h * DK : (h + 1) * DK]
        for it in range(ITERS):
            for dc in range(2):
                fbm[:, 2 + it * 2 + dc] = gcn_b[it, dc * 128 : (dc + 1) * 128]
        fbm[:, 6] = 1.0 if rh == 0 else 0.0
        fbm[:, 7] = 0.0 if rh == 0 else 1.0

        maps.append(
            {
                "nodes": nodes,
                "wb": np.ascontiguousarray(wbm.astype(bf16)),
                "fb": np.ascontiguousarray(fbm),
            }
        )
    return maps


def kernel(trace=False, tmpdir=None, **inputs):
    from concourse.bass_utils import run_bass_kernel_spmd

    nc = _get_nc()
    maps = _in_maps(inputs)
    kw = {}
    if trace:
        kw = dict(trace=True, tmpdir=tmpdir)
    res = run_bass_kernel_spmd(nc, maps, core_ids=list(range(8)), **kw)

    agg_b = np.asarray(inputs["agg_b"], dtype=np.float32)
    out = np.zeros((B, N, D), np.float32)
    for b in range(B):
        for rh in range(2):
            rows = slice(rh * RH, (rh + 1) * RH)
            out[b, rows, :] = (
                res.results[4 * b + 0 * 2 + rh]["part"]
                + res.results[4 * b + 1 * 2 + rh]["part"]
                + agg_b
            )
    if trace:
        return out, res
    return out


# revision 29
# speedup vs baseline: 1.2611x; 1.0277x over previous
"""Trainium2 Bass kernel for nn_AttentionGCNLayer (B=2, N=4096, D=256, H=2, ITERS=2).

Sharding: 8 cores = (b in 2) x (h in 2) x (row-half in 2). Each core handles one
(batch, head) pair and one half (2048) of the attention rows, with a pairwise
AllGather of the updated node features between the two GCN iterations.

Schedule (v2): chunk-paced pipeline. The Scalar engine's exp stream (64
activations of [128,1024], ~73us) is the second wall after the PE (~114us);
the program interleaves PE work (scores, h-gen, R rowsums, agg accumulation)
at neighbor-chunk granularity so the PE is never idle waiting on exp:

  A:  kq gen + scores(mega-tile 0) exp-paced, h1 + R(rt0) + agg(rt0)
      partial accumulation interleaved per chunk-pair.
  B:  scores(mega-tile 1) feeding exp(1), R(rt1)/agg(rt1) blocks, then
      chunk-paced R(rt2)/agg(rt2) against the exp(1) drain; h2 + AllGather
      of updated features fire per row-pair as soon as x1 is ready.
  D:  iter-2 aggregation (pair order: cc0-half first, local, cc1-half last
      to match AllGather arrival), output projection + DMA interleaved.

Layout: x kept transposed (x^T [D on 2x128 partitions, N free]) in local row
order; scores computed transposed (E^T = exp(q k^T)^T, neighbors on
partitions) feeding the aggregation matmuls directly; E and h in fp8 with
DoubleRow matmuls; weights arrive pre-cast to bf16 from the host in a single
packed blob (no on-device staging casts). Softmax normalizer R = rowsum(E)
via DoubleRow ones-matmuls; 1/R via the fast DVE reciprocal. Scalar engine
runs exp only; copies/relu/bias live on the Vector engine.
"""

import sys

if "/opt/trn_rl_repo" not in sys.path:
    sys.path.insert(0, "/opt/trn_rl_repo")

import numpy as np

B, N, D, H, ITERS = 2, 4096, 256, 2, 2
DK = D // H                      # 128
RH = N // 2                      # 2048 rows per core
NCH = N // 128                   # 32 neighbor chunks
NCP = NCH // 2                   # 16 neighbor chunk-pairs
RT = 512                         # row tile (one PSUM bank of fp32)
NRT = RH // RT                   # 4 row tiles per core
SCALE = 1.0 / float(np.sqrt(np.float32(DK)))

# packed bf16 weight blob column offsets
WQ0, WK0, GW0, AGG0 = 0, 256, 512, 1536
WBCOLS = 2048

_CACHE = {}


def _seq_engines(mybir):
    return {
        mybir.EngineType.PE,
        mybir.EngineType.Activation,
        mybir.EngineType.Pool,
        mybir.EngineType.DVE,
        mybir.EngineType.SP,
    }


def _split_excess_waits(nc, mybir, max_waits=1):
    """This container's walrus accepts at most one sync-wait per engine
    instruction; hoist extra waits onto preceding NoOps on the same engine."""
    seq = _seq_engines(mybir)
    n_new = 0
    for f in nc.m.functions:
        for blk in f.blocks:
            if not any(
                inst.sync_info is not None
                and inst.sync_info.on_wait
                and len(inst.sync_info.on_wait) > max_waits
                and inst.engine in seq
                for inst in blk.instructions
            ):
                continue
            out = []
            for inst in blk.instructions:
                si = inst.sync_info
                if (
                    si is not None
                    and si.on_wait
                    and len(si.on_wait) > max_waits
                    and inst.engine in seq
                ):
                    waits = list(si.on_wait)
                    keep, extra = waits[:max_waits], waits[max_waits:]
                    while extra:
                        chunk, extra = extra[:max_waits], extra[max_waits:]
                        out.append(
                            mybir.InstNoOp(
                                name=f"{inst.name}-ws{n_new}",
                                sync_info=mybir.SyncInfo(on_wait=chunk, on_update=[]),
                                bass_nofuse=True,
                                engine=inst.engine,
                            )
                        )
                        n_new += 1
                    inst.sync_info = mybir.SyncInfo(
                        on_wait=keep, on_update=list(si.on_update)
                    )
                out.append(inst)
            blk.instructions = out
    return n_new


def _build():
    import concourse.bass as bass
    import concourse.mybir as mybir
    import concourse.tile as tile

    f32 = mybir.dt.float32
    bf16 = mybir.dt.bfloat16
    fp8 = mybir.dt.float8e4
    AF = mybir.ActivationFunctionType
    ALU = mybir.AluOpType

    nc = bass.Bass("TRN2", num_devices=8)

    nodes = nc.dram_tensor("nodes", [D, N], bf16, kind="ExternalInput")
    wb = nc.dram_tensor("wb", [128, WBCOLS], bf16, kind="ExternalInput")
    fb = nc.dram_tensor("fb", [128, 8], f32, kind="ExternalInput")
    part = nc.dram_tensor("part", [RH, D], f32, kind="ExternalOutput")

    with tile.TileContext(nc) as tc:
        from contextlib import ExitStack

        with ExitStack() as ctx:
            const = ctx.enter_context(tc.tile_pool(name="const", bufs=1))

            ones_col = const.tile([128, 2, 16], fp8, name="ones_col")
            nc.vector.memset(ones_col, 1.0)
            ones_row = const.tile([1, 128], f32, name="ones_row")
            nc.vector.memset(ones_row, 1.0)

            # persistent state
            xT = [
                [
                    const.tile([128, RH], bf16, name=f"xT{dc}{hf}")
                    for hf in range(2)
                ]
                for dc in range(2)
            ]
            eP = [const.tile([128, 2, RH], fp8, name=f"eP{i}") for i in range(NCP)]
            hP = [const.tile([128, 2, D], fp8, name=f"hP{i}") for i in range(NCP)]
            hP2 = [const.tile([128, 2, D], fp8, name=f"hQ{i}") for i in range(NCP)]
            rinvB = const.tile([128, RH], f32, name="rinvB")

            wb_s = const.tile([128, WBCOLS], bf16, name="wb_s")
            fb_s = const.tile([128, 8], f32, name="fb_s")
            kT = const.tile([128, N], bf16, name="kT")
            qT = const.tile([128, RH], bf16, name="qT")

            def wq_sl(dc):
                return wb_s[:, WQ0 + dc * 128 : WQ0 + (dc + 1) * 128]

            def wk_sl(dc):
                return wb_s[:, WK0 + dc * 128 : WK0 + (dc + 1) * 128]

            def gw_sl(it, dc):
                o = GW0 + (it * 2 + dc) * 256
                return wb_s[:, o : o + 256]

            def agg_sl(dc):
                o = AGG0 + dc * 256
                return wb_s[:, o : o + 256]

            wqb_s = fb_s[:, 0:1]
            wkb_s = fb_s[:, 1:2]

            def gb_sl(it, dc):
                return fb_s[:, 2 + it * 2 + dc : 3 + it * 2 + dc]

            m0_s = fb_s[:, 6:7]
            m1_s = fb_s[:, 7:8]

            # weight + bias loads on the gpsimd DMA queue; nodes on sync
            nc.gpsimd.dma_start(out=wb_s, in_=wb[:, :])
            nc.gpsimd.dma_start(out=fb_s, in_=fb[:, :])

            # phase pools
            ps_u = ctx.enter_context(tc.tile_pool(name="ps_u", bufs=2, space="PSUM"))
            ps_h = ctx.enter_context(tc.tile_pool(name="ps_h", bufs=1, space="PSUM"))
            racc = ctx.enter_context(tc.tile_pool(name="racc", bufs=2))
            upd = ctx.enter_context(tc.tile_pool(name="upd", bufs=4))
            dram = ctx.enter_context(tc.tile_pool(name="dram", bufs=1, space="DRAM"))
            cct = ctx.enter_context(tc.tile_pool(name="cct", bufs=8))

            cc_in = [
                dram.tile([4 * 128, 2 * D], fp8, name=f"cc_in{g}") for g in range(2)
            ]
            cc_out = [
                dram.tile([8 * 128, 2 * D], fp8, name=f"cc_out{g}") for g in range(2)
            ]

            def load_quarter(q):
                # all quarters sequentially on the sync queue: per-queue
                # descriptors run in order, so quarter 0 completes first
                # instead of round-robining with the later quarters
                hf, base = (q // 2, (q % 2) * 1024)
                for dc in range(2):
                    nc.sync.dma_start(
                        out=xT[dc][hf][:, base : base + 1024],
                        in_=nodes[
                            dc * 128 : (dc + 1) * 128, q * 1024 : (q + 1) * 1024
                        ],
                    )

            def kq_gen(ps_k, wsl, bias_s, dst, hf, col):
                ps = ps_k.tile([128, RT], f32, name="psk", tag="psk")
                for dc in range(2):
                    nc.tensor.matmul(
                        ps,
                        wsl(dc),
                        xT[dc][hf][:, col : col + RT],
                        start=(dc == 0),
                        stop=(dc == 1),
                    )
                dcol = hf * RH + col
                nc.vector.tensor_scalar_add(
                    out=dst[:, dcol : dcol + RT], in0=ps, scalar1=bias_s
                )

            def sc(mt, ncx):
                # one neighbor chunk of transposed scores for mega-rowtile mt,
                # exp'ed into eP on the Scalar engine
                ps = ps_sc.tile([128, 2 * RT], f32, name="pss", tag="pss")
                for j in range(2):
                    nc.tensor.matmul(
                        ps[:, j * RT : (j + 1) * RT],
                        kT[:, ncx * 128 : (ncx + 1) * 128],
                        qT[:, (2 * mt + j) * RT : (2 * mt + j + 1) * RT],
                        start=True,
                        stop=True,
                    )
                nc.scalar.activation(
                    out=eP[ncx // 2][:, ncx % 2, 2 * mt * RT : (2 * mt + 2) * RT],
                    in_=ps,
                    func=AF.Exp,
                    scale=SCALE,
                )

            def h_chunk(it, ncx):
                hf, col = (0, ncx * 128) if ncx < NCP else (1, (ncx - NCP) * 128)
                ps = ps_h.tile([128, D], f32, name="psh", tag="psh")
                for dc in range(2):
                    nc.tensor.matmul(
                        ps,
                        xT[dc][hf][:, col : col + 128],
                        gw_sl(it, dc),
                        start=(dc == 0),
                        stop=(dc == 1),
                    )
                hdst = hP if it == 0 else hP2
                nc.vector.tensor_copy(out=hdst[ncx // 2][:, ncx % 2, :], in_=ps)

            def r_alloc():
                return ps_r.tile([1, RT], f32, name="psrow", tag="psr")

            def r_step(ps_row, rt, cp, start, stop):
                nc.tensor.matmul(
                    ps_row,
                    ones_col[:, :, 0:1],
                    eP[cp][:, :, rt * RT : (rt + 1) * RT],
                    start=start,
                    stop=stop,
                    perf_mode=mybir.MatmulPerfMode.DoubleRow,
                )

            def r_fin(ps_row, rt):
                # broadcast R across partitions on the PE, then 1/x on DVE
                # (the custom-DVE fast reciprocal doesn't codegen in this
                # toolchain)
                rrow = racc.tile([1, RT], f32, name="rrow", tag="rrow")
                nc.vector.tensor_copy(out=rrow, in_=ps_row)
                ps_b = ps_r.tile([128, RT], f32, name="psb", tag="psr")
                nc.tensor.matmul(ps_b, ones_row, rrow, start=True, stop=True)
                nc.vector.reciprocal(
                    out=rinvB[:, rt * RT : (rt + 1) * RT], in_=ps_b
                )

            def agg_alloc():
                return [
                    ps_u.tile([128, RT], f32, name=f"pu{dc}", tag="pu")
                    for dc in range(2)
                ]

            def agg_step(pu, it, rt, cp, start, stop):
                hx = hP if it == 0 else hP2
                for dc in range(2):
                    nc.tensor.matmul(
                        pu[dc],
                        hx[cp][:, :, dc * 128 : (dc + 1) * 128],
                        eP[cp][:, :, rt * RT : (rt + 1) * RT],
                        start=start,
                        stop=stop,
                        perf_mode=mybir.MatmulPerfMode.DoubleRow,
                    )

            def upd_fin(pu, it, rt):
                # x += relu(agg/R + b): mul, fused bias+relu, residual add (DVE)
                for dc in range(2):
                    t = upd.tile([128, RT], f32, name="updt", tag="updt")
                    nc.vector.tensor_mul(
                        t, pu[dc], rinvB[:, rt * RT : (rt + 1) * RT]
                    )
                    nc.vector.tensor_scalar(
                        out=t,
                        in0=t,
                        scalar1=gb_sl(it, dc),
                        scalar2=0.0,
                        op0=ALU.add,
                        op1=ALU.max,
                    )
                    nc.vector.tensor_add(
                        out=xT[dc][0][:, rt * RT : (rt + 1) * RT],
                        in0=xT[dc][0][:, rt * RT : (rt + 1) * RT],
                        in1=t,
                    )

            def h2_dma(rt):
                # stage this rowtile's h2 pair-tiles into the exchange buffer
                for i, cp in enumerate((2 * rt, 2 * rt + 1)):
                    nc.sync.dma_start(
                        out=cc_in[rt // 2][
                            ((rt % 2) * 2 + i) * 128 : ((rt % 2) * 2 + i + 1) * 128,
                            :,
                        ],
                        in_=hP2[cp][:, :, :].rearrange("p a b -> p (a b)"),
                    )

            def fire_cc(g):
                nc.gpsimd.collective_compute(
                    "AllGather",
                    mybir.AluOpType.bypass,
                    replica_groups=[[0, 1], [2, 3], [4, 5], [6, 7]],
                    ins=[cc_in[g][:, :].opt()],
                    outs=[cc_out[g][:, :].opt()],
                )

            def combine(g):
                # place partner h2 pair-tiles into hP2[8+4g .. 12+4g];
                # rank-select via the m0/m1 input masks (2 fused DVE ops)
                for i in range(4):
                    t0 = cct.tile([128, 2 * D], fp8, name="t0", tag="cct")
                    t1 = cct.tile([128, 2 * D], fp8, name="t1", tag="cct")
                    nc.sync.dma_start(
                        out=t0, in_=cc_out[g][i * 128 : (i + 1) * 128, :]
                    )
                    nc.sync.dma_start(
                        out=t1, in_=cc_out[g][(4 + i) * 128 : (5 + i) * 128, :]
                    )
                    nc.vector.tensor_scalar_mul(t0, t0, m1_s)
                    nc.vector.scalar_tensor_tensor(
                        out=hP2[8 + 4 * g + i][:, :, :].rearrange("p a b -> p (a b)"),
                        in0=t1,
                        scalar=m0_s,
                        in1=t0,
                        op0=ALU.mult,
                        op1=ALU.add,
                    )

            # ---------------- phase A ----------------
            # loads + all kq gen (own PSUM pool, closed before scores pools
            # open), then chunk-paced: scores(0) / h1 / R(rt0) / agg0(rt0)
            # interleaved against the exp(0) drain
            load_quarter(0)
            load_quarter(1)
            load_quarter(2)
            load_quarter(3)
            with tc.tile_pool(name="ps_k", bufs=3, space="PSUM") as ps_k:
                for q in range(2):
                    base = q * 1024
                    for ct in range(2):
                        kq_gen(ps_k, wk_sl, wkb_s, kT, 0, base + ct * RT)
                        kq_gen(ps_k, wq_sl, wqb_s, qT, 0, base + ct * RT)
                for q in range(2):
                    base = q * 1024
                    for ct in range(2):
                        kq_gen(ps_k, wk_sl, wkb_s, kT, 1, base + ct * RT)

            p2 = ExitStack()
            ps_r = p2.enter_context(tc.tile_pool(name="ps_r", bufs=1, space="PSUM"))
            p1 = ExitStack()
            ps_sc = p1.enter_context(tc.tile_pool(name="ps_sc", bufs=2, space="PSUM"))

            pr0 = r_alloc()
            pu0 = agg_alloc()

            def ab_tail(c):
                # interleaved consumers trailing the exp stream by 4 chunks
                if c >= 4 and c % 2 == 0:
                    cp = (c - 4) // 2
                    r_step(pr0, 0, cp, start=(cp == 0), stop=False)
                    agg_step(pu0, 0, 0, cp, start=(cp == 0), stop=False)

            for c in range(32):
                sc(0, c)
                h_chunk(0, c)
                ab_tail(c)
            # drain rt0 pair-steps cp=14,15 and finish
            for cp in (14, 15):
                r_step(pr0, 0, cp, start=False, stop=(cp == 15))
                agg_step(pu0, 0, 0, cp, start=False, stop=(cp == 15))
            r_fin(pr0, 0)
            upd_fin(pu0, 0, 0)

            # ---------------- phase B ----------------
            # scores(1) feeds exp(1) continuously (PE work here sized to the
            # Scalar pace); rt1 blocks run on exp(0), rt2 is chunk-paced
            # against the exp(1) drain; h2 + the first AllGather fire as
            # soon as x1 of rowtiles 0/1 exists
            for c in range(0, 2):
                sc(1, c)
            pr1 = r_alloc()
            for cp in range(NCP):
                r_step(pr1, 1, cp, start=(cp == 0), stop=(cp == NCP - 1))
            r_fin(pr1, 1)
            for c in range(2, 4):
                sc(1, c)
            pu1 = agg_alloc()
            for c in range(4, 8):
                sc(1, c)
                agg_step(pu1, 0, 1, 2 * (c - 4), start=(c == 4), stop=False)
                agg_step(pu1, 0, 1, 2 * (c - 4) + 1, start=False, stop=False)
            # h2 of rowtile 0 (upd(rt0)'s DVE chain has had time to drain)
            for c in range(8, 10):
                sc(1, c)
                h_chunk(1, 2 * (c - 8))
                h_chunk(1, 2 * (c - 8) + 1)
            h2_dma(0)
            for c in range(10, 14):
                sc(1, c)
                agg_step(pu1, 0, 1, 2 * (c - 6), start=False, stop=False)
                agg_step(
                    pu1, 0, 1, 2 * (c - 6) + 1, start=False,
                    stop=(c == 13),
                )
            upd_fin(pu1, 0, 1)
            for c in range(14, 16):
                sc(1, c)
                h_chunk(1, 4 + 2 * (c - 14))
                h_chunk(1, 4 + 2 * (c - 14) + 1)
            h2_dma(1)
            fire_cc(0)
            # chunk-paced rt2 agg + R(rt2) against the exp(1) drain
            # (paced pairs 0..7, drained pairs 8..15)
            pr2 = r_alloc()
            pu2 = agg_alloc()
            for c in range(16, 32):
                sc(1, c)
                if c % 2 == 1:
                    cp = (c - 17) // 2
                    r_step(pr2, 2, cp, start=(cp == 0), stop=False)
                    agg_step(pu2, 0, 2, cp, start=(cp == 0), stop=False)
            for cp in range(8, NCP):
                r_step(pr2, 2, cp, start=False, stop=(cp == NCP - 1))
                agg_step(pu2, 0, 2, cp, start=False, stop=(cp == NCP - 1))
            r_fin(pr2, 2)
            upd_fin(pu2, 0, 2)
            p1.close()
            p2.close()

            # ---------------- phase C (post-exp) ----------------
            # agg(rt3) + R(rt3) on freshly opened pools (no DVE gates); h2
            # of rowtiles 2/3 and the start of the iter-2 aggregation cover
            # the DVE update chains; the second exchange fires as early as
            # its dependencies allow
            ps_u2 = ctx.enter_context(tc.tile_pool(name="ps_u2", bufs=2, space="PSUM"))

            def agg_alloc2(pool, nm):
                return [
                    pool.tile([128, RT], f32, name=f"{nm}{dc}", tag="pu")
                    for dc in range(2)
                ]

            pu3 = agg_alloc2(ps_u2, "pw")
            with tc.tile_pool(name="ps_c", bufs=1, space="PSUM") as ps_c:
                pr3 = ps_c.tile([1, RT], f32, name="psrow3", tag="psc")
                for cp in range(NCP):
                    agg_step(pu3, 0, 3, cp, start=(cp == 0), stop=(cp == NCP - 1))
                    r_step(pr3, 3, cp, start=(cp == 0), stop=(cp == NCP - 1))
                rrow3 = racc.tile([1, RT], f32, name="rrow", tag="rrow")
                nc.vector.tensor_copy(out=rrow3, in_=pr3)
                psb3 = ps_c.tile([128, RT], f32, name="psb3", tag="psc")
                nc.tensor.matmul(psb3, ones_row, rrow3, start=True, stop=True)
                nc.vector.reciprocal(out=rinvB[:, 3 * RT : 4 * RT], in_=psb3)
                for ncx in range(8, 12):
                    h_chunk(1, ncx)
                h2_dma(2)
                upd_fin(pu3, 0, 3)
                # start iter-2 aggregation (pass 1: local pairs of rowtile
                # 0) to keep the PE busy while the DVE finishes upd(rt3)
                pv0 = agg_alloc2(ps_u, "pv")
                for cp in range(8):
                    agg_step(pv0, 1, 0, cp, start=(cp == 0), stop=False)
                for ncx in range(12, 16):
                    h_chunk(1, ncx)
                h2_dma(3)
                fire_cc(1)
                combine(0)

            # ---------------- phase D ----------------
            # two-pass iter-2 aggregation: pass 1 = local + cc0 pairs into
            # an SBUF partial (already scaled by 1/R); pass 2 = cc1 pairs
            # once the second AllGather lands. No rowtile ever waits on
            # cc(1) with PSUM held.
            pso = ctx.enter_context(tc.tile_pool(name="pso", bufs=3, space="PSUM"))
            ost = ctx.enter_context(tc.tile_pool(name="ost", bufs=4))
            pp = ctx.enter_context(tc.tile_pool(name="pp", bufs=1))
            combine(1)

            def pass1(rt, pu=None, skip=0):
                if pu is None:
                    pu = agg_alloc2(ps_u if rt % 2 == 0 else ps_u2, "pv")
                for i in range(skip, 12):
                    agg_step(pu, 1, rt, i, start=(i == 0), stop=(i == 11))
                parts = []
                for dc in range(2):
                    pt = pp.tile([128, RT], f32, name=f"pp{rt}{dc}", tag=f"pp{rt}{dc}")
                    nc.vector.tensor_mul(
                        pt, pu[dc], rinvB[:, rt * RT : (rt + 1) * RT]
                    )
                    parts.append(pt)
                return parts

            def pass2(rt, parts):
                pu = agg_alloc2(ps_u if rt % 2 == 0 else ps_u2, "pv")
                for i, cp in enumerate(range(12, 16)):
                    agg_step(pu, 1, rt, cp, start=(i == 0), stop=(i == 3))
                for dc in range(2):
                    t = upd.tile([128, RT], f32, name="updt", tag="updt")
                    nc.vector.tensor_mul(
                        t, pu[dc], rinvB[:, rt * RT : (rt + 1) * RT]
                    )
                    nc.vector.tensor_add(out=t, in0=t, in1=parts[dc])
                    nc.vector.tensor_scalar(
                        out=t,
                        in0=t,
                        scalar1=gb_sl(1, dc),
                        scalar2=0.0,
                        op0=ALU.add,
                        op1=ALU.max,
                    )
                    nc.vector.tensor_add(
                        out=xT[dc][0][:, rt * RT : (rt + 1) * RT],
                        in0=xT[dc][0][:, rt * RT : (rt + 1) * RT],
                        in1=t,
                    )

            def out_chunk(rc, qi):
                ps = pso.tile([128, D], f32, name="pso", tag="pso")
                for dc in range(2):
                    nc.tensor.matmul(
                        ps,
                        xT[dc][0][:, rc * 128 : (rc + 1) * 128],
                        agg_sl(dc),
                        start=(dc == 0),
                        stop=(dc == 1),
                    )
                ot = ost.tile([128, D], f32, name="ot", tag="ot")
                nc.vector.tensor_copy(out=ot, in_=ps)
                eng = nc.sync if qi % 2 == 0 else nc.scalar
                eng.dma_start(out=part[rc * 128 : (rc + 1) * 128, :], in_=ot)

            parts = [pass1(0, pu=pv0, skip=8)]
            for rt in range(1, NRT):
                parts.append(pass1(rt))
            # skew the output projection one rowtile behind pass 2 so the
            # PE never waits on the DVE update chains
            for rt in range(NRT):
                pass2(rt, parts[rt])
                if rt > 0:
                    for rc in range(4 * (rt - 1), 4 * rt):
                        out_chunk(rc, rc)
            for rc in range(4 * (NRT - 1), 4 * NRT):
                out_chunk(rc, rc)

    _split_excess_waits(nc, mybir)
    return nc


def _get_nc():
    if "nc" not in _CACHE:
        _CACHE["nc"] = _build()
    return _CACHE["nc"]


def _in_maps(inputs):
    import ml_dtypes

    bf16 = ml_dtypes.bfloat16

    ne = np.asarray(inputs["nodes_embed"], dtype=np.float32)
    wq_w = np.asarray(inputs["WQ_w"], dtype=np.float32)
    wq_b = np.asarray(inputs["WQ_b"], dtype=np.float32)
    wk_w = np.asarray(inputs["WK_w"], dtype=np.float32)
    wk_b = np.asarray(inputs["WK_b"], dtype=np.float32)
    gcn_w = np.asarray(inputs["gcn_W"], dtype=np.float32)
    gcn_b = np.asarray(inputs["gcn_b"], dtype=np.float32)
    agg_w = np.asarray(inputs["agg_W"], dtype=np.float32)

    maps = []
    for c in range(8):
        b, h, rh = c // 4, (c // 2) % 2, c % 2
        if rh == 0:
            nodes = ne[b]
        else:
            nodes = np.concatenate([ne[b, RH:], ne[b, :RH]], axis=0)
        nodes = np.ascontiguousarray(nodes.T).astype(bf16)  # [D, N], x^T

        wq_h = wq_w[:, h * DK : (h + 1) * DK]
        wk_h = wk_w[:, h * DK : (h + 1) * DK]
        agg_h = agg_w[h * D : (h + 1) * D, :]
        wbm = np.zeros((128, WBCOLS), np.float32)
        wbm[:, WQ0 : WQ0 + 128] = wq_h[0:128, :]
        wbm[:, WQ0 + 128 : WQ0 + 256] = wq_h[128:256, :]
        wbm[:, WK0 : WK0 + 128] = wk_h[0:128, :]
        wbm[:, WK0 + 128 : WK0 + 256] = wk_h[128:256, :]
        for it in range(ITERS):
            for dc in range(2):
                o = GW0 + (it * 2 + dc) * 256
                wbm[:, o : o + 256] = gcn_w[it, dc * 128 : (dc + 1) * 128, :]
        for dc in range(2):
            o = AGG0 + dc * 256
            wbm[:, o : o + 256] = agg_h[dc * 128 : (dc + 1) * 128, :]

        fbm = np.zeros((128, 8), np.float32)
        fbm[:, 0] = wq_b[h * DK : (h + 1) * DK]
        fbm[:, 1] = wk_b[h * DK : (h + 1) * DK]
        for it in range(ITERS):
            for dc in range(2):
                fbm[:, 2 + it * 2 + dc] = gcn_b[it, dc * 128 : (dc + 1) * 128]
        fbm[:, 6] = 1.0 if rh == 0 else 0.0
        fbm[:, 7] = 0.0 if rh == 0 else 1.0

        maps.append(
            {
                "nodes": nodes,
                "wb": np.ascontiguousarray(wbm.astype(bf16)),
                "fb": np.ascontiguousarray(fbm),
            }
        )
    return maps


def kernel(trace=False, tmpdir=None, **inputs):
    from concourse.bass_utils import run_bass_kernel_spmd

    nc = _get_nc()
    maps = _in_maps(inputs)
    kw = {}
    if trace:
        kw = dict(trace=True, tmpdir=tmpdir)
    res = run_bass_kernel_spmd(nc, maps, core_ids=list(range(8)), **kw)

    agg_b = np.asarray(inputs["agg_b"], dtype=np.float32)
    out = np.zeros((B, N, D), np.float32)
    for b in range(B):
        for rh in range(2):
            rows = slice(rh * RH, (rh + 1) * RH)
            out[b, rows, :] = (
                res.results[4 * b + 0 * 2 + rh]["part"]
                + res.results[4 * b + 1 * 2 + rh]["part"]
                + agg_b
            )
    if trace:
        return out, res
    return out


# revision 39
# speedup vs baseline: 1.3001x; 1.0310x over previous
"""Trainium2 Bass kernel for nn_AttentionGCNLayer (B=2, N=4096, D=256, H=2, ITERS=2).

Sharding: 8 cores = (b in 2) x (h in 2) x (row-half in 2). Each core handles one
(batch, head) pair and one half (2048) of the attention rows, with a pairwise
AllGather of the updated node features between the two GCN iterations.

Schedule (v2): chunk-paced pipeline. The Scalar engine's exp stream (64
activations of [128,1024], ~73us) is the second wall after the PE (~114us);
the program interleaves PE work (scores, h-gen, R rowsums, agg accumulation)
at neighbor-chunk granularity so the PE is never idle waiting on exp:

  A:  kq gen + scores(mega-tile 0) exp-paced, h1 + R(rt0) + agg(rt0)
      partial accumulation interleaved per chunk-pair.
  B:  scores(mega-tile 1) feeding exp(1), R(rt1)/agg(rt1) blocks, then
      chunk-paced R(rt2)/agg(rt2) against the exp(1) drain; h2 + AllGather
      of updated features fire per row-pair as soon as x1 is ready.
  D:  iter-2 aggregation (pair order: cc0-half first, local, cc1-half last
      to match AllGather arrival), output projection + DMA interleaved.

Layout: x kept transposed (x^T [D on 2x128 partitions, N free]) in local row
order; scores computed transposed (E^T = exp(q k^T)^T, neighbors on
partitions) feeding the aggregation matmuls directly; E and h in fp8 with
DoubleRow matmuls; weights arrive pre-cast to bf16 from the host in a single
packed blob (no on-device staging casts). Softmax normalizer R = rowsum(E)
via DoubleRow ones-matmuls; 1/R via the fast DVE reciprocal. Scalar engine
runs exp only; copies/relu/bias live on the Vector engine.
"""

import sys

if "/opt/trn_rl_repo" not in sys.path:
    sys.path.insert(0, "/opt/trn_rl_repo")

import numpy as np

B, N, D, H, ITERS = 2, 4096, 256, 2, 2
DK = D // H                      # 128
RH = N // 2                      # 2048 rows per core
NCH = N // 128                   # 32 neighbor chunks
NCP = NCH // 2                   # 16 neighbor chunk-pairs
RT = 512                         # row tile (one PSUM bank of fp32)
NRT = RH // RT                   # 4 row tiles per core
SCALE = 1.0 / float(np.sqrt(np.float32(DK)))

# packed bf16 weight blob column offsets
WQ0, WK0, GW0, AGG0 = 0, 256, 512, 1536
WBCOLS = 2048

_CACHE = {}


def _seq_engines(mybir):
    return {
        mybir.EngineType.PE,
        mybir.EngineType.Activation,
        mybir.EngineType.Pool,
        mybir.EngineType.DVE,
        mybir.EngineType.SP,
    }


def _split_excess_waits(nc, mybir, max_waits=1):
    """This container's walrus accepts at most one sync-wait per engine
    instruction; hoist extra waits onto preceding NoOps on the same engine."""
    seq = _seq_engines(mybir)
    n_new = 0
    for f in nc.m.functions:
        for blk in f.blocks:
            if not any(
                inst.sync_info is not None
                and inst.sync_info.on_wait
                and len(inst.sync_info.on_wait) > max_waits
                and inst.engine in seq
                for inst in blk.instructions
            ):
                continue
            out = []
            for inst in blk.instructions:
                si = inst.sync_info
                if (
                    si is not None
                    and si.on_wait
                    and len(si.on_wait) > max_waits
                    and inst.engine in seq
                ):
                    waits = list(si.on_wait)
                    keep, extra = waits[:max_waits], waits[max_waits:]
                    while extra:
                        chunk, extra = extra[:max_waits], extra[max_waits:]
                        out.append(
                            mybir.InstNoOp(
                                name=f"{inst.name}-ws{n_new}",
                                sync_info=mybir.SyncInfo(on_wait=chunk, on_update=[]),
                                bass_nofuse=True,
                                engine=inst.engine,
                            )
                        )
                        n_new += 1
                    inst.sync_info = mybir.SyncInfo(
                        on_wait=keep, on_update=list(si.on_update)
                    )
                out.append(inst)
            blk.instructions = out
    return n_new


def _build():
    import concourse.bass as bass
    import concourse.mybir as mybir
    import concourse.tile as tile

    f32 = mybir.dt.float32
    bf16 = mybir.dt.bfloat16
    fp8 = mybir.dt.float8e4
    AF = mybir.ActivationFunctionType
    ALU = mybir.AluOpType

    nc = bass.Bass("TRN2", num_devices=8)

    nodes = nc.dram_tensor("nodes", [D, N], bf16, kind="ExternalInput")
    wb = nc.dram_tensor("wb", [128, WBCOLS], bf16, kind="ExternalInput")
    fb = nc.dram_tensor("fb", [128, 8], f32, kind="ExternalInput")
    part = nc.dram_tensor("part", [RH, D], f32, kind="ExternalOutput")

    with tile.TileContext(nc) as tc:
        from contextlib import ExitStack

        with ExitStack() as ctx:
            const = ctx.enter_context(tc.tile_pool(name="const", bufs=1))

            ones_col = const.tile([128, 2, 16], fp8, name="ones_col")
            nc.vector.memset(ones_col, 1.0)
            ones_row = const.tile([1, 128], f32, name="ones_row")
            nc.vector.memset(ones_row, 1.0)

            # persistent state
            xT = [
                [
                    const.tile([128, RH], bf16, name=f"xT{dc}{hf}")
                    for hf in range(2)
                ]
                for dc in range(2)
            ]
            eP = [const.tile([128, 2, RH], fp8, name=f"eP{i}") for i in range(NCP)]
            hP = [const.tile([128, 2, D], fp8, name=f"hP{i}") for i in range(NCP)]
            hP2 = [const.tile([128, 2, D], fp8, name=f"hQ{i}") for i in range(NCP)]
            rinvB = const.tile([128, RH], f32, name="rinvB")

            wb_s = const.tile([128, WBCOLS], bf16, name="wb_s")
            fb_s = const.tile([128, 8], f32, name="fb_s")
            kT = const.tile([128, N], bf16, name="kT")
            qT = const.tile([128, RH], bf16, name="qT")

            def wq_sl(dc):
                return wb_s[:, WQ0 + dc * 128 : WQ0 + (dc + 1) * 128]

            def wk_sl(dc):
                return wb_s[:, WK0 + dc * 128 : WK0 + (dc + 1) * 128]

            def gw_sl(it, dc):
                o = GW0 + (it * 2 + dc) * 256
                return wb_s[:, o : o + 256]

            def agg_sl(dc):
                o = AGG0 + dc * 256
                return wb_s[:, o : o + 256]

            wqb_s = fb_s[:, 0:1]
            wkb_s = fb_s[:, 1:2]

            def gb_sl(it, dc):
                return fb_s[:, 2 + it * 2 + dc : 3 + it * 2 + dc]

            m0_s = fb_s[:, 6:7]
            m1_s = fb_s[:, 7:8]

            # weight + bias loads on the gpsimd DMA queue; nodes on sync
            nc.gpsimd.dma_start(out=wb_s, in_=wb[:, :])
            nc.gpsimd.dma_start(out=fb_s, in_=fb[:, :])

            # phase pools
            ps_u = ctx.enter_context(tc.tile_pool(name="ps_u", bufs=2, space="PSUM"))
            ps_h = ctx.enter_context(tc.tile_pool(name="ps_h", bufs=1, space="PSUM"))
            racc = ctx.enter_context(tc.tile_pool(name="racc", bufs=2))
            upd = ctx.enter_context(tc.tile_pool(name="upd", bufs=4))
            dram = ctx.enter_context(tc.tile_pool(name="dram", bufs=1, space="DRAM"))
            cct = ctx.enter_context(tc.tile_pool(name="cct", bufs=8))

            cc_in = [
                dram.tile([4 * 128, 2 * D], fp8, name=f"cc_in{g}") for g in range(2)
            ]
            cc_out = [
                dram.tile([8 * 128, 2 * D], fp8, name=f"cc_out{g}") for g in range(2)
            ]

            def load_quarter(q):
                # all quarters sequentially on the sync queue: per-queue
                # descriptors run in order, so quarter 0 completes first
                # instead of round-robining with the later quarters
                hf, base = (q // 2, (q % 2) * 1024)
                for dc in range(2):
                    nc.sync.dma_start(
                        out=xT[dc][hf][:, base : base + 1024],
                        in_=nodes[
                            dc * 128 : (dc + 1) * 128, q * 1024 : (q + 1) * 1024
                        ],
                    )

            def kq_gen(ps_k, wsl, bias_s, dst, hf, col):
                # tag "pss" so late kq calls can ride the scores pool's ring
                ps = ps_k.tile([128, RT], f32, name="psk", tag="pss")
                for dc in range(2):
                    nc.tensor.matmul(
                        ps,
                        wsl(dc),
                        xT[dc][hf][:, col : col + RT],
                        start=(dc == 0),
                        stop=(dc == 1),
                    )
                dcol = hf * RH + col
                nc.vector.tensor_scalar_add(
                    out=dst[:, dcol : dcol + RT], in0=ps, scalar1=bias_s
                )

            def sc(mt, ncx):
                # one neighbor chunk of transposed scores for mega-rowtile mt,
                # exp'ed into eP on the Scalar engine
                ps = ps_sc.tile([128, 2 * RT], f32, name="pss", tag="pss")
                for j in range(2):
                    nc.tensor.matmul(
                        ps[:, j * RT : (j + 1) * RT],
                        kT[:, ncx * 128 : (ncx + 1) * 128],
                        qT[:, (2 * mt + j) * RT : (2 * mt + j + 1) * RT],
                        start=True,
                        stop=True,
                    )
                nc.scalar.activation(
                    out=eP[ncx // 2][:, ncx % 2, 2 * mt * RT : (2 * mt + 2) * RT],
                    in_=ps,
                    func=AF.Exp,
                    scale=SCALE,
                )

            def h_chunk(it, ncx):
                hf, col = (0, ncx * 128) if ncx < NCP else (1, (ncx - NCP) * 128)
                ps = ps_h.tile([128, D], f32, name="psh", tag="psh")
                for dc in range(2):
                    nc.tensor.matmul(
                        ps,
                        xT[dc][hf][:, col : col + 128],
                        gw_sl(it, dc),
                        start=(dc == 0),
                        stop=(dc == 1),
                    )
                hdst = hP if it == 0 else hP2
                nc.vector.tensor_copy(out=hdst[ncx // 2][:, ncx % 2, :], in_=ps)

            def r_alloc():
                return ps_r.tile([1, RT], f32, name="psrow", tag="psr")

            def r_step(ps_row, rt, cp, start, stop):
                nc.tensor.matmul(
                    ps_row,
                    ones_col[:, :, 0:1],
                    eP[cp][:, :, rt * RT : (rt + 1) * RT],
                    start=start,
                    stop=stop,
                    perf_mode=mybir.MatmulPerfMode.DoubleRow,
                )

            def r_fin(ps_row, rt):
                # broadcast R across partitions on the PE, then 1/x on DVE
                # (the custom-DVE fast reciprocal doesn't codegen in this
                # toolchain)
                rrow = racc.tile([1, RT], f32, name="rrow", tag="rrow")
                nc.vector.tensor_copy(out=rrow, in_=ps_row)
                ps_b = ps_r.tile([128, RT], f32, name="psb", tag="psr")
                nc.tensor.matmul(ps_b, ones_row, rrow, start=True, stop=True)
                nc.vector.reciprocal(
                    out=rinvB[:, rt * RT : (rt + 1) * RT], in_=ps_b
                )

            def agg_alloc():
                return [
                    ps_u.tile([128, RT], f32, name=f"pu{dc}", tag="pu")
                    for dc in range(2)
                ]

            def agg_step(pu, it, rt, cp, start, stop):
                hx = hP if it == 0 else hP2
                for dc in range(2):
                    nc.tensor.matmul(
                        pu[dc],
                        hx[cp][:, :, dc * 128 : (dc + 1) * 128],
                        eP[cp][:, :, rt * RT : (rt + 1) * RT],
                        start=start,
                        stop=stop,
                        perf_mode=mybir.MatmulPerfMode.DoubleRow,
                    )

            def upd_fin(pu, it, rt, scalar_relu=False):
                # x += relu(agg/R + b): mul, fused bias+relu, residual add.
                # relu on DVE while the Scalar engine runs exp; on Scalar
                # once exp is done.
                for dc in range(2):
                    t = upd.tile([128, RT], f32, name="updt", tag="updt")
                    nc.vector.tensor_mul(
                        t, pu[dc], rinvB[:, rt * RT : (rt + 1) * RT]
                    )
                    if scalar_relu:
                        nc.scalar.activation(
                            out=t, in_=t, func=AF.Relu,
                            bias=gb_sl(it, dc), scale=1.0,
                        )
                    else:
                        nc.vector.tensor_scalar(
                            out=t,
                            in0=t,
                            scalar1=gb_sl(it, dc),
                            scalar2=0.0,
                            op0=ALU.add,
                            op1=ALU.max,
                        )
                    nc.vector.tensor_add(
                        out=xT[dc][0][:, rt * RT : (rt + 1) * RT],
                        in0=xT[dc][0][:, rt * RT : (rt + 1) * RT],
                        in1=t,
                    )

            def h2_dma(rt):
                # stage this rowtile's h2 pair-tiles into the exchange buffer
                for i, cp in enumerate((2 * rt, 2 * rt + 1)):
                    nc.sync.dma_start(
                        out=cc_in[rt // 2][
                            ((rt % 2) * 2 + i) * 128 : ((rt % 2) * 2 + i + 1) * 128,
                            :,
                        ],
                        in_=hP2[cp][:, :, :].rearrange("p a b -> p (a b)"),
                    )

            def fire_cc(g):
                nc.gpsimd.collective_compute(
                    "AllGather",
                    mybir.AluOpType.bypass,
                    replica_groups=[[0, 1], [2, 3], [4, 5], [6, 7]],
                    ins=[cc_in[g][:, :].opt()],
                    outs=[cc_out[g][:, :].opt()],
                )

            def combine(g):
                # place partner h2 pair-tiles into hP2[8+4g .. 12+4g];
                # rank-select via the m0/m1 input masks (2 fused DVE ops)
                for i in range(4):
                    t0 = cct.tile([128, 2 * D], fp8, name="t0", tag="cct")
                    t1 = cct.tile([128, 2 * D], fp8, name="t1", tag="cct")
                    nc.sync.dma_start(
                        out=t0, in_=cc_out[g][i * 128 : (i + 1) * 128, :]
                    )
                    nc.sync.dma_start(
                        out=t1, in_=cc_out[g][(4 + i) * 128 : (5 + i) * 128, :]
                    )
                    nc.vector.tensor_scalar_mul(t0, t0, m1_s)
                    nc.vector.scalar_tensor_tensor(
                        out=hP2[8 + 4 * g + i][:, :, :].rearrange("p a b -> p (a b)"),
                        in0=t1,
                        scalar=m0_s,
                        in1=t0,
                        op0=ALU.mult,
                        op1=ALU.add,
                    )

            # ---------------- phase A ----------------
            # loads + all kq gen (own PSUM pool, closed before scores pools
            # open), then chunk-paced: scores(0) / h1 / R(rt0) / agg0(rt0)
            # interleaved against the exp(0) drain
            load_quarter(0)
            load_quarter(1)
            load_quarter(2)
            load_quarter(3)
            with tc.tile_pool(name="ps_k", bufs=3, space="PSUM") as ps_k:
                for q in range(2):
                    base = q * 1024
                    for ct in range(2):
                        kq_gen(ps_k, wk_sl, wkb_s, kT, 0, base + ct * RT)
                        kq_gen(ps_k, wq_sl, wqb_s, qT, 0, base + ct * RT)

            p2 = ExitStack()
            ps_r = p2.enter_context(tc.tile_pool(name="ps_r", bufs=1, space="PSUM"))
            p1 = ExitStack()
            ps_sc = p1.enter_context(tc.tile_pool(name="ps_sc", bufs=2, space="PSUM"))

            pr0 = r_alloc()
            pu0 = agg_alloc()

            def ab_tail(c):
                # interleaved consumers trailing the exp stream by 4 chunks
                if c >= 4 and c % 2 == 0:
                    cp = (c - 4) // 2
                    r_step(pr0, 0, cp, start=(cp == 0), stop=False)
                    agg_step(pu0, 0, 0, cp, start=(cp == 0), stop=False)

            for c in range(32):
                if c == 8:
                    # kq for row-half 1 (k only), on the scores PSUM ring
                    for q in range(2):
                        for ct in range(2):
                            kq_gen(ps_sc, wk_sl, wkb_s, kT, 1, q * 1024 + ct * RT)
                sc(0, c)
                h_chunk(0, c)
                ab_tail(c)
            # prime the mega-tile-1 scores stream before the A-drain so the
            # Scalar engine rolls straight from exp(0) into exp(1)
            sc(1, 0)
            sc(1, 1)
            # drain rt0 pair-steps cp=14,15 and finish
            for cp in (14, 15):
                r_step(pr0, 0, cp, start=False, stop=(cp == 15))
                agg_step(pu0, 0, 0, cp, start=False, stop=(cp == 15))
            sc(1, 2)
            sc(1, 3)
            r_fin(pr0, 0)
            upd_fin(pu0, 0, 0)

            # ---------------- phase B ----------------
            # scores(1) feeds exp(1) continuously (PE work here sized to the
            # Scalar pace); rt1 blocks run on exp(0), rt2 is chunk-paced
            # against the exp(1) drain; h2 + the first AllGather fire as
            # soon as x1 of rowtiles 0/1 exists
            pr1 = r_alloc()
            for cp in range(NCP):
                r_step(pr1, 1, cp, start=(cp == 0), stop=(cp == NCP - 1))
            r_fin(pr1, 1)
            pu1 = agg_alloc()
            for c in range(4, 8):
                sc(1, c)
                agg_step(pu1, 0, 1, 2 * (c - 4), start=(c == 4), stop=False)
                agg_step(pu1, 0, 1, 2 * (c - 4) + 1, start=False, stop=False)
            # h2 of rowtile 0 (upd(rt0)'s DVE chain has had time to drain)
            for c in range(8, 10):
                sc(1, c)
                h_chunk(1, 2 * (c - 8))
                h_chunk(1, 2 * (c - 8) + 1)
            h2_dma(0)
            for c in range(10, 14):
                sc(1, c)
                agg_step(pu1, 0, 1, 2 * (c - 6), start=False, stop=False)
                agg_step(
                    pu1, 0, 1, 2 * (c - 6) + 1, start=False,
                    stop=(c == 13),
                )
            upd_fin(pu1, 0, 1)
            for c in range(14, 16):
                sc(1, c)
                h_chunk(1, 4 + 2 * (c - 14))
                h_chunk(1, 4 + 2 * (c - 14) + 1)
            h2_dma(1)
            fire_cc(0)
            # chunk-paced rt2 agg + R(rt2) against the exp(1) drain
            # (paced pairs 0..7, drained pairs 8..15)
            pr2 = r_alloc()
            pu2 = agg_alloc()
            for c in range(16, 32):
                sc(1, c)
                if c % 2 == 1:
                    cp = (c - 17) // 2
                    r_step(pr2, 2, cp, start=(cp == 0), stop=False)
                    agg_step(pu2, 0, 2, cp, start=(cp == 0), stop=False)
            for cp in range(8, NCP):
                r_step(pr2, 2, cp, start=False, stop=(cp == NCP - 1))
                agg_step(pu2, 0, 2, cp, start=False, stop=(cp == NCP - 1))
            r_fin(pr2, 2)
            upd_fin(pu2, 0, 2)
            p1.close()
            p2.close()

            # ---------------- phase C (post-exp) ----------------
            # agg(rt3) + R(rt3) on freshly opened pools (no DVE gates); h2
            # of rowtiles 2/3 and the start of the iter-2 aggregation cover
            # the DVE update chains; the second exchange fires as early as
            # its dependencies allow
            ps_u2 = ctx.enter_context(tc.tile_pool(name="ps_u2", bufs=2, space="PSUM"))

            def agg_alloc2(pool, nm):
                return [
                    pool.tile([128, RT], f32, name=f"{nm}{dc}", tag="pu")
                    for dc in range(2)
                ]

            pu3 = agg_alloc2(ps_u2, "pw")
            with tc.tile_pool(name="ps_c", bufs=1, space="PSUM") as ps_c:
                pr3 = ps_c.tile([1, RT], f32, name="psrow3", tag="psc")
                for cp in range(NCP):
                    agg_step(pu3, 0, 3, cp, start=(cp == 0), stop=(cp == NCP - 1))
                    r_step(pr3, 3, cp, start=(cp == 0), stop=(cp == NCP - 1))
                rrow3 = racc.tile([1, RT], f32, name="rrow", tag="rrow")
                nc.vector.tensor_copy(out=rrow3, in_=pr3)
                psb3 = ps_c.tile([128, RT], f32, name="psb3", tag="psc")
                nc.tensor.matmul(psb3, ones_row, rrow3, start=True, stop=True)
                nc.vector.reciprocal(out=rinvB[:, 3 * RT : 4 * RT], in_=psb3)
                for ncx in range(8, 12):
                    h_chunk(1, ncx)
                h2_dma(2)
                upd_fin(pu3, 0, 3, scalar_relu=True)
                # start iter-2 aggregation (pass 1: local pairs of rowtile
                # 0) to keep the PE busy while the DVE finishes upd(rt3)
                pv0 = agg_alloc2(ps_u, "pv")
                for cp in range(8):
                    agg_step(pv0, 1, 0, cp, start=(cp == 0), stop=False)
                for ncx in range(12, 16):
                    h_chunk(1, ncx)
                h2_dma(3)
                fire_cc(1)
                combine(0)

            # ---------------- phase D ----------------
            # two-pass iter-2 aggregation: pass 1 = local + cc0 pairs into
            # an SBUF partial (already scaled by 1/R); pass 2 = cc1 pairs
            # once the second AllGather lands. No rowtile ever waits on
            # cc(1) with PSUM held.
            pso = ctx.enter_context(tc.tile_pool(name="pso", bufs=3, space="PSUM"))
            ost = ctx.enter_context(tc.tile_pool(name="ost", bufs=4))
            pp = ctx.enter_context(tc.tile_pool(name="pp", bufs=1))
            combine(1)

            def pass1(rt, pu=None, skip=0):
                if pu is None:
                    pu = agg_alloc2(ps_u if rt % 2 == 0 else ps_u2, "pv")
                for i in range(skip, 12):
                    agg_step(pu, 1, rt, i, start=(i == 0), stop=(i == 11))
                parts = []
                for dc in range(2):
                    pt = pp.tile([128, RT], f32, name=f"pp{rt}{dc}", tag=f"pp{rt}{dc}")
                    nc.vector.tensor_mul(
                        pt, pu[dc], rinvB[:, rt * RT : (rt + 1) * RT]
                    )
                    parts.append(pt)
                return parts

            def pass2(rt, parts):
                pu = agg_alloc2(ps_u if rt % 2 == 0 else ps_u2, "pv")
                for i, cp in enumerate(range(12, 16)):
                    agg_step(pu, 1, rt, cp, start=(i == 0), stop=(i == 3))
                for dc in range(2):
                    t = upd.tile([128, RT], f32, name="updt", tag="updt")
                    nc.vector.tensor_mul(
                        t, pu[dc], rinvB[:, rt * RT : (rt + 1) * RT]
                    )
                    nc.vector.tensor_add(out=t, in0=t, in1=parts[dc])
                    # relu+bias on the Scalar engine (idle once exp is done)
                    nc.scalar.activation(
                        out=t, in_=t, func=AF.Relu, bias=gb_sl(1, dc), scale=1.0
                    )
                    nc.vector.tensor_add(
                        out=xT[dc][0][:, rt * RT : (rt + 1) * RT],
                        in0=xT[dc][0][:, rt * RT : (rt + 1) * RT],
                        in1=t,
                    )

            def out_chunk(rc, qi):
                ps = pso.tile([128, D], f32, name="pso", tag="pso")
                for dc in range(2):
                    nc.tensor.matmul(
                        ps,
                        xT[dc][0][:, rc * 128 : (rc + 1) * 128],
                        agg_sl(dc),
                        start=(dc == 0),
                        stop=(dc == 1),
                    )
                ot = ost.tile([128, D], f32, name="ot", tag="ot")
                nc.scalar.copy(out=ot, in_=ps)
                eng = nc.sync if qi % 2 == 0 else nc.gpsimd
                eng.dma_start(out=part[rc * 128 : (rc + 1) * 128, :], in_=ot)

            parts = [pass1(0, pu=pv0, skip=8)]
            for rt in range(1, NRT):
                parts.append(pass1(rt))
            # skew the output projection one rowtile behind pass 2 so the
            # PE never waits on the DVE update chains
            for rt in range(NRT):
                pass2(rt, parts[rt])
                if rt > 0:
                    for rc in range(4 * (rt - 1), 4 * rt):
                        out_chunk(rc, rc)
            for rc in range(4 * (NRT - 1), 4 * NRT):
                out_chunk(rc, rc)

    _split_excess_waits(nc, mybir)
    return nc


def _get_nc():
    if "nc" not in _CACHE:
        _CACHE["nc"] = _build()
    return _CACHE["nc"]


def _in_maps(inputs):
    import ml_dtypes

    bf16 = ml_dtypes.bfloat16

    ne = np.asarray(inputs["nodes_embed"], dtype=np.float32)
    wq_w = np.asarray(inputs["WQ_w"], dtype=np.float32)
    wq_b = np.asarray(inputs["WQ_b"], dtype=np.float32)
    wk_w = np.asarray(inputs["WK_w"], dtype=np.float32)
    wk_b = np.asarray(inputs["WK_b"], dtype=np.float32)
    gcn_w = np.asarray(inputs["gcn_W"], dtype=np.float32)
    gcn_b = np.asarray(inputs["gcn_b"], dtype=np.float32)
    agg_w = np.asarray(inputs["agg_W"], dtype=np.float32)

    maps = []
    for c in range(8):
        b, h, rh = c // 4, (c // 2) % 2, c % 2
        if rh == 0:
            nodes = ne[b]
        else:
            nodes = np.concatenate([ne[b, RH:], ne[b, :RH]], axis=0)
        nodes = np.ascontiguousarray(nodes.T).astype(bf16)  # [D, N], x^T

        wq_h = wq_w[:, h * DK : (h + 1) * DK]
        wk_h = wk_w[:, h * DK : (h + 1) * DK]
        agg_h = agg_w[h * D : (h + 1) * D, :]
        wbm = np.zeros((128, WBCOLS), np.float32)
        wbm[:, WQ0 : WQ0 + 128] = wq_h[0:128, :]
        wbm[:, WQ0 + 128 : WQ0 + 256] = wq_h[128:256, :]
        wbm[:, WK0 : WK0 + 128] = wk_h[0:128, :]
        wbm[:, WK0 + 128 : WK0 + 256] = wk_h[128:256, :]
        for it in range(ITERS):
            for dc in range(2):
                o = GW0 + (it * 2 + dc) * 256
                wbm[:, o : o + 256] = gcn_w[it, dc * 128 : (dc + 1) * 128, :]
        for dc in range(2):
            o = AGG0 + dc * 256
            wbm[:, o : o + 256] = agg_h[dc * 128 : (dc + 1) * 128, :]

        fbm = np.zeros((128, 8), np.float32)
        fbm[:, 0] = wq_b[h * DK : (h + 1) * DK]
        fbm[:, 1] = wk_b[h * DK : (h + 1) * DK]
        for it in range(ITERS):
            for dc in range(2):
                fbm[:, 2 + it * 2 + dc] = gcn_b[it, dc * 128 : (dc + 1) * 128]
        fbm[:, 6] = 1.0 if rh == 0 else 0.0
        fbm[:, 7] = 0.0 if rh == 0 else 1.0

        maps.append(
            {
                "nodes": nodes,
                "wb": np.ascontiguousarray(wbm.astype(bf16)),
                "fb": np.ascontiguousarray(fbm),
            }
        )
    return maps


def kernel(trace=False, tmpdir=None, **inputs):
    from concourse.bass_utils import run_bass_kernel_spmd

    nc = _get_nc()
    maps = _in_maps(inputs)
    kw = {}
    if trace:
        kw = dict(trace=True, tmpdir=tmpdir)
    res = run_bass_kernel_spmd(nc, maps, core_ids=list(range(8)), **kw)

    agg_b = np.asarray(inputs["agg_b"], dtype=np.float32)
    out = np.zeros((B, N, D), np.float32)
    for b in range(B):
        for rh in range(2):
            rows = slice(rh * RH, (rh + 1) * RH)
            out[b, rows, :] = (
                res.results[4 * b + 0 * 2 + rh]["part"]
                + res.results[4 * b + 1 * 2 + rh]["part"]
                + agg_b
            )
    if trace:
        return out, res
    return out
